# revision 1
# baseline (speedup 1.0000x reference)
"""Trainium2 Bass kernel for nn_AttentionBlock (sparse_attention, no-softmax).

Computation (per batch b):
    qh = (q @ Wq^T) split into 16 heads of dk=64     [S, D] -> [H, S, DK]
    kh, vh likewise
    scores = (qh @ kh^T) / sqrt(DK)                  [H, S, S]
    p      = scores * A^T                            (elementwise structural mask)
    x      = p @ vh                                  [H, S, DK] -> [S, D]
    out    = x @ Wo^T + bo                           [S, D]

Sharding over 8 NeuronCores: data-parallel over batch (B=2) x tensor-parallel
over heads (16 heads -> 4 per core). Each core projects q/k/v for its 4 heads
(column-parallel), runs masked attention for them, and applies its 256-column
slice of the output projection (row-parallel), producing a full-shape partial
output. Host sums the 4 partials per batch.

Implementation notes:
- Activations are shipped pre-transposed ([D, S]) so every matmul contraction
  dim lands on SBUF partitions with no on-device transposes; 1/sqrt(DK) is
  folded into the mask A on the host.
- The whole data path runs in fp16 with fp32 PSUM accumulation (all operands
  here are O(1)-O(100), well inside fp16 range; measured end-to-end error is
  a few 1e-4). fp16 is the same PE stream rate as bf16/f32r but, being
  2-byte, additionally halves DMA/SBUF traffic and legalizes PE quadrant
  packing (tile_position), which f32/f32r reject.
- Heads are stored as pairs on the partition axis (head 2j on partitions
  0:63, head 2j+1 on 64:127). The K=64 score matmuls of a pair run
  concurrently in the upper/lower PE row-quadrants (tile_position (0,0) /
  (64,0)); the M=64 p@v matmuls of a pair run concurrently in left/right
  col-quadrants into one PSUM bank (tile_position (0,0) / (0,64)).
- The mask multiply is the throughput-critical elementwise stage; it is
  spread over three engines: DVE (straight out of PSUM), and a ScalarE
  PSUM->SBUF bounce feeding GPSIMD, alternating per key-block.
- Projection work for the next/previous query block is interleaved into the
  attention loop so no engine drains the pipeline at block boundaries.
"""

import numpy as np

import concourse.mybir as mybir
import concourse.tile as tile
from concourse import bacc, bass_utils

B, S, D, H = 2, 2048, 1024, 16
NCORES = 8
GROUPS = NCORES // B          # 4 head-groups
HPC = H // GROUPS             # 4 heads per core
DK = D // H                   # 64
HD = HPC * DK                 # 256 head-dim columns per core
NPAIR = HPC // 2              # 2 head pairs per core
SCALE = 1.0 / np.sqrt(DK)

P = 128                       # SBUF partitions
QB = 512                      # query block
NQB = S // QB                 # 4
KBLK = 128                    # key block
NKB = S // KBLK               # 16
NKT = D // P                  # 8 contraction chunks for projections
AGRP = 4                      # key-blocks per A-tile DMA / interleave group
NGRP = NKB // AGRP            # 4 groups

f32 = mybir.dt.float32
f16 = mybir.dt.float16

_CACHED = None  # built module, reused across kernel() calls
TRACE = False         # set True (e.g. from test.py) to profile the NEFF
LAST_RESULTS = None   # BassKernelResults of the most recent run


def _build():
    nc = bacc.Bacc("TRN2", target_bir_lowering=False)

    qT = nc.dram_tensor("qT", [D, S], f16, kind="ExternalInput")
    kT = nc.dram_tensor("kT", [D, S], f16, kind="ExternalInput")
    vT = nc.dram_tensor("vT", [D, S], f16, kind="ExternalInput")
    Asc = nc.dram_tensor("Asc", [S, S], f16, kind="ExternalInput")
    wq = nc.dram_tensor("wq", [D, HD], f16, kind="ExternalInput")
    wk = nc.dram_tensor("wk", [D, HD], f16, kind="ExternalInput")
    wv = nc.dram_tensor("wv", [D, HD], f16, kind="ExternalInput")
    wo = nc.dram_tensor("wo", [HD, D], mybir.dt.float32r, kind="ExternalInput")
    out = nc.dram_tensor("out", [S, D], f32, kind="ExternalOutput")

    qT_r = qT.rearrange("(kt p) s -> p kt s", p=P)
    kT_r = kT.rearrange("(kt p) s -> p kt s", p=P)
    vT_r = vT.rearrange("(kt p) s -> p kt s", p=P)
    wq_r = wq.rearrange("(kt p) c -> p kt c", p=P)
    wk_r = wk.rearrange("(kt p) c -> p kt c", p=P)
    wv_r = wv.rearrange("(kt p) c -> p kt c", p=P)
    wo_r = wo.rearrange("(ck p) e -> p ck e", p=P)
    A_r = Asc.rearrange("(kb p) q -> p kb q", p=P)

    with tile.TileContext(nc) as tc:
        with (
            tc.tile_pool(name="persist", bufs=1) as pp,
            tc.tile_pool(name="stream", bufs=2) as sp,
            tc.tile_pool(name="psU", bufs=2, space="PSUM") as psU,   # proj [128,512] x2 + sc x4
            tc.tile_pool(name="psX", bufs=1, space="PSUM") as psX,   # xT accumulators
        ):
            wk_sb = pp.tile([P, NKT, HD], f16, tag="wk")
            wv_sb = pp.tile([P, NKT, HD], f16, tag="wv")
            wq_sb = pp.tile([P, NKT, HD], f16, tag="wq")
            wo_sb = pp.tile([P, HD // P, D], mybir.dt.float32r, tag="wo")

            # head-PAIR layout: pair j holds head 2j on partitions 0:64 and
            # head 2j+1 on 64:128 — the layout quadrant packing requires
            khT_sb = pp.tile([P, NPAIR, S], f16, tag="khT")
            vh_sb = pp.tile([P, NKB, HD], f16, tag="vh")     # [ks%128, kb, c]

            def kv_proj_chains(st, kT_pre=None, vT_pre=None):
                """Issue k/v DMAs for s-slice st now; return per-chain thunks
                so the projection matmuls can be sprinkled between attention
                iterations instead of lumped at group boundaries."""
                sl = slice(st * QB, (st + 1) * QB)
                kT_sb = kT_pre
                if kT_sb is None:
                    kT_sb = sp.tile([P, NKT, QB], f16, tag="xin", bufs=4, name="kT_sb")
                    nc.sync.dma_start(kT_sb[:], kT_r[:, :, sl])
                vT_sb = vT_pre
                if vT_sb is None:
                    vT_sb = sp.tile([P, NKT, QB], f16, tag="xin", bufs=4, name="vT_sb")
                    nc.sync.dma_start(vT_sb[:], vT_r[:, :, sl])

                def kchain(ct, kT_sb=kT_sb):
                    pk = psU.tile([P, QB], f32, tag="u", name="pk")
                    for kt in range(NKT):
                        nc.tensor.matmul(
                            pk[:], wk_sb[:, kt, ct * P:(ct + 1) * P], kT_sb[:, kt, :],
                            start=(kt == 0), stop=(kt == NKT - 1),
                        )
                    # pair layout: both copies stay partition-aligned
                    nc.scalar.copy(khT_sb[0:DK, ct, sl], pk[0:DK, :])
                    nc.scalar.copy(khT_sb[DK:P, ct, sl], pk[DK:P, :])

                def vchain(ssub, vT_sb=vT_sb):
                    kb = st * (QB // P) + ssub
                    pv = psU.tile([P, HD], f32, tag="u", name="pv")
                    for kt in range(NKT):
                        nc.tensor.matmul(
                            pv[:], vT_sb[:, kt, ssub * P:(ssub + 1) * P], wv_sb[:, kt, :],
                            start=(kt == 0), stop=(kt == NKT - 1),
                        )
                    nc.scalar.copy(vh_sb[:, kb, :], pv[:])

                return [lambda ct=ct: kchain(ct) for ct in range(NPAIR)] + \
                       [lambda s=s: vchain(s) for s in range(QB // P)]

            def kv_proj(st, kT_pre=None, vT_pre=None):
                for thunk in kv_proj_chains(st, kT_pre, vT_pre):
                    thunk()

            def q_proj_chains(qb):
                """Issue the q DMA now; return (qhT tile, per-pair chain thunks)."""
                qsl = slice(qb * QB, (qb + 1) * QB)
                qT_sb = sp.tile([P, NKT, QB], f16, tag="xin", bufs=4, name="qT_sb")
                nc.sync.dma_start(qT_sb[:], qT_r[:, :, qsl])
                qhT_sb = sp.tile([P, NPAIR, QB], f16, tag="qh", bufs=3, name="qhT_sb")

                def chain(ct):
                    pq = psU.tile([P, QB], f32, tag="u", name="pq")
                    for kt in range(NKT):
                        nc.tensor.matmul(
                            pq[:], wq_sb[:, kt, ct * P:(ct + 1) * P], qT_sb[:, kt, :],
                            start=(kt == 0), stop=(kt == NKT - 1),
                        )
                    nc.scalar.copy(qhT_sb[0:DK, ct, :], pq[0:DK, :])
                    nc.scalar.copy(qhT_sb[DK:P, ct, :], pq[DK:P, :])

                return qhT_sb, [lambda ct=ct: chain(ct) for ct in range(NPAIR)]

            def q_proj(qb):
                qhT_sb, chains = q_proj_chains(qb)
                for thunk in chains:
                    thunk()
                return qhT_sb

            def o_proj_chain(xts, qb, ssub, ptag="u", pbufs=None):
                """One 128-row slice of the output projection for query block qb."""
                osb = sp.tile([P, D], f32, tag="osb", bufs=4, name="osb")
                for et in range(D // QB):
                    po = psU.tile([P, QB], f32, tag=ptag, bufs=pbufs, name="po")
                    for ck in range(HD // P):
                        nc.tensor.matmul(
                            po[:],
                            xts[:, ck, ssub * P:(ssub + 1) * P],
                            wo_sb[:, ck, et * QB:(et + 1) * QB],
                            start=(ck == 0), stop=(ck == HD // P - 1),
                        )
                    nc.scalar.copy(osb[:, et * QB:(et + 1) * QB], po[:])
                nc.sync.dma_start(
                    out[qb * QB + ssub * P:qb * QB + (ssub + 1) * P, :], osb[:]
                )

            # ---- pipeline ---------------------------------------------------------
            # prologue DMAs in dependency-first order so the PE starts ASAP
            nc.sync.dma_start(wk_sb[:, 0:NKT // 2, :], wk_r[:, 0:NKT // 2, :])
            nc.sync.dma_start(wk_sb[:, NKT // 2:, :], wk_r[:, NKT // 2:, :])
            kT0 = sp.tile([P, NKT, QB], f16, tag="xin", bufs=4, name="kT_sb")
            nc.sync.dma_start(kT0[:, 0:NKT // 2, :], kT_r[:, 0:NKT // 2, 0:QB])
            nc.sync.dma_start(kT0[:, NKT // 2:, :], kT_r[:, NKT // 2:, 0:QB])
            nc.sync.dma_start(wv_sb[:], wv_r[:])
            vT0 = sp.tile([P, NKT, QB], f16, tag="xin", bufs=4, name="vT_sb")
            nc.sync.dma_start(vT0[:], vT_r[:, :, 0:QB])
            nc.sync.dma_start(wq_sb[:], wq_r[:])
            A0 = sp.tile([P, AGRP, QB], f16, tag="A", bufs=3, name="A_sb")
            nc.sync.dma_start(A0[:], A_r[:, 0:AGRP, 0:QB])

            kv_proj(0, kT_pre=kT0, vT_pre=vT0)
            qhT_cur = q_proj(0)
            nc.sync.dma_start(wo_sb[:], wo_r[:])

            pend_xts = None    # (xts tile, qb) awaiting output projection
            qhT_next = None

            for qb in range(NQB):
                qsl = slice(qb * QB, (qb + 1) * QB)
                xt = psX.tile([P, NPAIR, QB], f32, tag="xt", name="xt")  # 2 banks
                xts = sp.tile([P, NPAIR, QB], mybir.dt.float32r, tag="xts", bufs=3, name="xts")
                pend_pts = None

                def emit_xt(pts, kb, xt=xt, xts=xts):
                    # p @ v: both heads of a pair run concurrently in the
                    # left/right PE col-quadrants into one PSUM bank.
                    # skip_group_check: the two col-quadrant groups legally
                    # share one PSUM bank (sim-only guard).
                    for j in range(NPAIR):
                        nc.tensor.matmul(
                            xt[0:DK, j, :],
                            vh_sb[:, kb, (2 * j) * DK:(2 * j + 1) * DK],
                            pts[2 * j],
                            start=(kb == 0), stop=(kb == NKB - 1),
                            tile_position=(0, 0), skip_group_check=True,
                        )
                        nc.tensor.matmul(
                            xt[DK:P, j, :],
                            vh_sb[:, kb, (2 * j + 1) * DK:(2 * j + 2) * DK],
                            pts[2 * j + 1],
                            start=(kb == 0), stop=(kb == NKB - 1),
                            tile_position=(0, DK), skip_group_check=True,
                        )
                        if kb == NKB - 1:
                            # drain this pair's accumulator immediately
                            nc.scalar.copy(xts[:, j, :], xt[:, j, :])

                fillers = []
                for kbg in range(NGRP):
                    if qb == 0 and kbg == 0:
                        A_sb = A0
                    else:
                        A_sb = sp.tile([P, AGRP, QB], f16, tag="A", bufs=3, name="A_sb")
                        nc.sync.dma_start(
                            A_sb[:], A_r[:, kbg * AGRP:(kbg + 1) * AGRP, qsl]
                        )
                    # queue this group's independent projection work; it is
                    # drained two chains per key-block below, keeping the PE
                    # fed without starving the elementwise engines
                    if qb == 0:
                        if kbg < NGRP - 1:
                            fillers += kv_proj_chains(kbg + 1)
                        else:
                            qhT_next, qchains = q_proj_chains(1)
                            fillers += qchains
                    else:
                        if kbg < 2 and pend_xts is not None:
                            xts_p, qb_p = pend_xts
                            fillers.append(
                                lambda x=xts_p, q=qb_p, s=2 * kbg: o_proj_chain(x, q, s))
                            fillers.append(
                                lambda x=xts_p, q=qb_p, s=2 * kbg + 1: o_proj_chain(x, q, s))
                            if kbg == 1:
                                pend_xts = None
                        elif kbg == NGRP - 1 and qb < NQB - 1:
                            qhT_next, qchains = q_proj_chains(qb + 1)
                            fillers += qchains
                    for i in range(AGRP):
                        kb = kbg * AGRP + i
                        ksl = slice(kb * KBLK, (kb + 1) * KBLK)
                        # scores: both heads of a pair run concurrently in the
                        # upper/lower PE row-quadrants
                        scs = []
                        for j in range(NPAIR):
                            sc_e = psU.tile([P, QB], f32, tag="sc", bufs=4, name="sc_e")
                            nc.tensor.matmul(
                                sc_e[:], khT_sb[0:DK, j, ksl], qhT_cur[0:DK, j, :],
                                start=True, stop=True, tile_position=(0, 0),
                            )
                            sc_o = psU.tile([P, QB], f32, tag="sc", bufs=4, name="sc_o")
                            nc.tensor.matmul(
                                sc_o[:], khT_sb[DK:P, j, ksl], qhT_cur[DK:P, j, :],
                                start=True, stop=True, tile_position=(DK, 0),
                            )
                            scs += [sc_e, sc_o]
                        # mask multiply, spread over DVE / (ScalarE+GPSIMD):
                        # heads 0,1 on DVE; heads 2,3 alternate by key-block
                        pts = []
                        for h in range(HPC):
                            pt = sp.tile([P, QB], f16, tag="pt", bufs=12, name="pt")
                            use_gp = h == 3 or (h == 2 and kb % 4 == 0)
                            if use_gp:
                                sc_sb = sp.tile([P, QB], f32, tag="scb", bufs=6,
                                                name="sc_sb")
                                nc.scalar.copy(sc_sb[:], scs[h][:])
                                nc.gpsimd.tensor_tensor(
                                    pt[:], sc_sb[:], A_sb[:, i, :],
                                    mybir.AluOpType.mult,
                                )
                            else:
                                nc.vector.tensor_tensor(
                                    pt[:], scs[h][:], A_sb[:, i, :],
                                    mybir.AluOpType.mult,
                                )
                            pts.append(pt)
                        # software pipeline: emit kb-1's p@v matmuls now, so the
                        # PE never waits mid-iteration for this kb's mask mults
                        if pend_pts is not None:
                            emit_xt(*pend_pts)
                        pend_pts = (pts, kb)
                        for _ in range(2):
                            if fillers:
                                fillers.pop(0)()


                while fillers:
                    fillers.pop(0)()
                emit_xt(*pend_pts)  # drain the last key-block
                pend_xts = (xts, qb)
                qhT_cur, qhT_next = qhT_next, None

            # drain the last query block's output projection through the
            # score banks (idle by now) for deeper tail pipelining
            xts_p, qb_p = pend_xts
            for ssub in range(QB // P):
                o_proj_chain(xts_p, qb_p, ssub, ptag="sc", pbufs=4)

    nc.compile()
    return nc


def _numpy_fallback(q, k, v, A, Wq, bq, Wk, bk, Wv, bv, Wo, bo):
    def proj(x, W, b):
        y = x @ W.T + b
        return y.reshape(B, S, H, DK).transpose(0, 2, 1, 3)

    qh, kh, vh = proj(q, Wq, bq), proj(k, Wk, bk), proj(v, Wv, bv)
    scores = np.einsum("bhqd,bhkd->bhqk", qh, kh) * np.float32(SCALE)
    p = scores * A.T
    x = np.einsum("bhqk,bhkd->bhqd", p, vh)
    x = x.transpose(0, 2, 1, 3).reshape(B, S, D)
    return (x @ Wo.T + bo).astype(np.float32)


def kernel(**inputs):
    q = np.asarray(inputs["q"], dtype=np.float32)
    k = np.asarray(inputs["k"], dtype=np.float32)
    v = np.asarray(inputs["v"], dtype=np.float32)
    A = np.asarray(inputs["A"], dtype=np.float32)
    Wq = np.asarray(inputs["Wq"], dtype=np.float32)
    Wk = np.asarray(inputs["Wk"], dtype=np.float32)
    Wv = np.asarray(inputs["Wv"], dtype=np.float32)
    Wo = np.asarray(inputs["Wo"], dtype=np.float32)
    bq, bk, bv, bo = (np.asarray(inputs[n], dtype=np.float32) for n in ("bq", "bk", "bv", "bo"))

    # The device kernel folds zero biases away (spec fills them with zeros);
    # fall back to a host reference in the (unused) nonzero-bias case.
    if any(np.any(b) for b in (bq, bk, bv)):
        return _numpy_fallback(q, k, v, A, Wq, bq, Wk, bk, Wv, bv, Wo, bo)

    global _CACHED
    if _CACHED is None:
        _CACHED = _build()
    nc = _CACHED

    Asc = np.ascontiguousarray((A * np.float32(SCALE)).astype(np.float16))
    in_maps = []
    for c in range(NCORES):
        b, g = divmod(c, GROUPS)
        hsl = slice(g * HD, (g + 1) * HD)
        in_maps.append({
            "qT": np.ascontiguousarray(q[b].T.astype(np.float16)),
            "kT": np.ascontiguousarray(k[b].T.astype(np.float16)),
            "vT": np.ascontiguousarray(v[b].T.astype(np.float16)),
            "Asc": Asc,
            "wq": np.ascontiguousarray(Wq[hsl].T.astype(np.float16)),
            "wk": np.ascontiguousarray(Wk[hsl].T.astype(np.float16)),
            "wv": np.ascontiguousarray(Wv[hsl].T.astype(np.float16)),
            "wo": np.ascontiguousarray(Wo[:, hsl].T),
        })

    res = bass_utils.run_bass_kernel_spmd(
        nc, in_maps, core_ids=list(range(NCORES)), trace=TRACE
    )
    global LAST_RESULTS
    LAST_RESULTS = res

    out = np.zeros((B, S, D), dtype=np.float32)
    for c in range(NCORES):
        out[c // GROUPS] += res.results[c]["out"]
    out += bo
    return out


if __name__ == "__main__":
    rng = np.random.default_rng(0)
    ins = {
        "q": rng.standard_normal((B, S, D), dtype=np.float32),
        "k": rng.standard_normal((B, S, D), dtype=np.float32),
        "v": rng.standard_normal((B, S, D), dtype=np.float32),
        "A": rng.random((S, S), dtype=np.float32),
        "Wq": rng.standard_normal((D, D), dtype=np.float32) / 32,
        "bq": np.zeros(D, np.float32),
        "Wk": rng.standard_normal((D, D), dtype=np.float32) / 32,
        "bk": np.zeros(D, np.float32),
        "Wv": rng.standard_normal((D, D), dtype=np.float32) / 32,
        "bv": np.zeros(D, np.float32),
        "Wo": rng.standard_normal((D, D), dtype=np.float32) / 32,
        "bo": np.zeros(D, np.float32),
    }
    got = kernel(**ins)
    ref = _numpy_fallback(**ins)
    err = np.abs(got - ref).max() / np.abs(ref).max()
    print("self-check relmax:", err)



# revision 17
# speedup vs baseline: 1.1088x; 1.1088x over previous
"""Trainium2 Bass kernel for nn_AttentionBlock (sparse_attention, no-softmax).

Computation (per batch b):
    qh = (q @ Wq^T) split into 16 heads of dk=64     [S, D] -> [H, S, DK]
    kh, vh likewise
    scores = (qh @ kh^T) / sqrt(DK)                  [H, S, S]
    p      = scores * A^T                            (elementwise structural mask)
    x      = p @ vh                                  [H, S, DK] -> [S, D]
    out    = x @ Wo^T + bo                           [S, D]

Sharding over 8 NeuronCores: data-parallel over batch (B=2) x tensor-parallel
over heads (16 heads -> 4 per core). Each core projects q/k/v for its 4 heads
(column-parallel), runs masked attention for them, and applies its 256-column
slice of the output projection (row-parallel), producing a full-shape partial
output. Host sums the 4 partials per batch.

Implementation notes:
- Activations ship pre-transposed ([D, S]); 1/sqrt(DK) is folded into the
  mask A on the host; whole data path in fp16 with fp32 PSUM accumulation.
- Heads are stored as pairs on the partition axis. Score matmuls of a pair
  run concurrently in the upper/lower PE row-quadrants into the TWO banks of
  one [128,2,512] PSUM tile; p@v matmuls run concurrently in left/right
  col-quadrants of one bank.
- The mask multiply processes a head-pair per instruction ([128,2,512], the
  A block shared across the pair via a stride-0 broadcast AP). Work is split
  three ways to fit under the PE time: DVE straight out of PSUM, an
  Activation PSUM->SBUF fp16 bounce feeding either GPSIMD or a 2x-mode DVE
  multiply (all-fp16-SBUF operands run at 2 elem/cycle on DVE).
- p@v consumption is software-pipelined 2 key-blocks behind the DVE-masked
  pair and 4 key-blocks behind the bounced pairs, hiding mask latency.
- Projection work for other blocks is interleaved into the attention loop;
  output is stored fp16 (partials summed on host in fp32).
"""

import numpy as np

import concourse.mybir as mybir
import concourse.tile as tile
from concourse import bacc, bass_utils

B, S, D, H = 2, 2048, 1024, 16
NCORES = 8
GROUPS = NCORES // B          # 4 head-groups
HPC = H // GROUPS             # 4 heads per core
DK = D // H                   # 64
HD = HPC * DK                 # 256 head-dim columns per core
NPAIR = HPC // 2              # 2 head pairs per core
SCALE = 1.0 / np.sqrt(DK)

P = 128                       # SBUF partitions
QB = 512                      # query block
NQB = S // QB                 # 4
KBLK = 128                    # key block
NKB = S // KBLK               # 16
NKT = D // P                  # 8 contraction chunks for projections
AGRP = 4                      # key-blocks per A-tile DMA
NGRP = NKB // AGRP            # 4 groups

f32 = mybir.dt.float32
f16 = mybir.dt.float16
f32r = mybir.dt.float32r

# per-kb mask path for the second head pair (first pair always DVE-from-PSUM).
# 'pool': Act bounce -> GPSIMD;  'dve2x': Act bounce -> DVE 2x;  'dve': DVE.
# The pool path has the longest latency, so it is confined to early key-blocks
# and its p@v consumption pended the deepest; late key-blocks use the fast
# paths so the end-of-block pipeline flush never waits on GPSIMD.
MODES_STEADY = ["pool", "dve2x", "pool", "pool",
                "pool", "dve2x", "pool", "pool",
                "pool", "pool", "pool", "pool",
                "dve2x", "pool", "dve2x", "dve"]
# qb0's PE span is projection-heavy (all of K/V): more slack for bounces
MODES_QB0 = ["pool", "pool", "pool", "pool",
             "pool", "pool", "dve2x", "pool",
             "pool", "pool", "pool", "pool",
             "pool", "dve2x", "dve2x", "dve"]
# last block: no pool in the final key-blocks so the tail flush+output
# projection never waits on GPSIMD latency
MODES_LAST = ["pool", "pool", "pool", "pool",
              "pool", "pool", "pool", "pool",
              "pool", "pool", "pool", "dve2x",
              "dve2x", "dve2x", "dve2x", "dve"]

_CACHED = None  # built module, reused across kernel() calls
TRACE = False         # set True (e.g. from test.py) to profile the NEFF
LAST_RESULTS = None   # BassKernelResults of the most recent run


def _build():
    nc = bacc.Bacc("TRN2", target_bir_lowering=False)

    qT = nc.dram_tensor("qT", [D, S], f16, kind="ExternalInput")
    kT = nc.dram_tensor("kT", [D, S], f16, kind="ExternalInput")
    vT = nc.dram_tensor("vT", [D, S], f16, kind="ExternalInput")
    Asc = nc.dram_tensor("Asc", [S, S], f16, kind="ExternalInput")
    wq = nc.dram_tensor("wq", [D, HD], f16, kind="ExternalInput")
    wk = nc.dram_tensor("wk", [D, HD], f16, kind="ExternalInput")
    wv = nc.dram_tensor("wv", [D, HD], f16, kind="ExternalInput")
    wo = nc.dram_tensor("wo", [HD, D], f32r, kind="ExternalInput")
    out = nc.dram_tensor("out", [S, D], f16, kind="ExternalOutput")

    qT_r = qT.rearrange("(kt p) s -> p kt s", p=P)
    kT_r = kT.rearrange("(kt p) s -> p kt s", p=P)
    vT_r = vT.rearrange("(kt p) s -> p kt s", p=P)
    wq_r = wq.rearrange("(kt p) c -> p kt c", p=P)
    wk_r = wk.rearrange("(kt p) c -> p kt c", p=P)
    wv_r = wv.rearrange("(kt p) c -> p kt c", p=P)
    wo_r = wo.rearrange("(ck p) e -> p ck e", p=P)
    A_r = Asc.rearrange("(kb p) q -> p kb q", p=P)

    with tile.TileContext(nc) as tc:
        with (
            tc.tile_pool(name="persist", bufs=1) as pp,
            tc.tile_pool(name="stream", bufs=2) as sp,
            tc.tile_pool(name="psU", bufs=2, space="PSUM") as psU,   # proj/oproj
            tc.tile_pool(name="psS", bufs=1, space="PSUM") as psS,   # score pairs
            tc.tile_pool(name="psX", bufs=1, space="PSUM") as psX,   # xT accum
        ):
            wk_sb = pp.tile([P, NKT, HD], f16, tag="wk")
            wv_sb = pp.tile([P, NKT, HD], f16, tag="wv")
            wq_sb = pp.tile([P, NKT, HD], f16, tag="wq")
            wo_sb = pp.tile([P, HD // P, D], f32r, tag="wo")

            # head-PAIR layout: pair j holds head 2j on partitions 0:64 and
            # head 2j+1 on 64:128
            khT_sb = pp.tile([P, NPAIR, S], f16, tag="khT")
            vh_sb = pp.tile([P, NKB, HD], f16, tag="vh")     # [ks%128, kb, c]

            # ---- interleavable projection work, split into ~850ns pieces ----
            # each piece is (cost_ns, thunk); PSUM tiles are allocated lazily
            # by the first piece of a chain and carried in a cell

            def kchain_pieces(ct, kT_sb, st):
                sl = slice(st * QB, (st + 1) * QB)
                cell = {}
                def half(h):
                    if h == 0:
                        cell["pk"] = psU.tile([P, QB], f32, tag="u", name="pk")
                    pk = cell["pk"]
                    for kt in range(h * NKT // 2, (h + 1) * NKT // 2):
                        nc.tensor.matmul(
                            pk[:], wk_sb[:, kt, ct * P:(ct + 1) * P], kT_sb[:, kt, :],
                            start=(kt == 0), stop=(kt == NKT - 1),
                        )
                    if h == 1:
                        nc.scalar.copy(khT_sb[:, ct, sl], pk[:])
                return [(853, lambda h=h: half(h)) for h in range(2)]

            def vchain_pieces(ssub, vT_sb, st):
                kb = st * (QB // P) + ssub
                def run():
                    pv = psU.tile([P, HD], f32, tag="u", name="pv")
                    for kt in range(NKT):
                        nc.tensor.matmul(
                            pv[:], vT_sb[:, kt, ssub * P:(ssub + 1) * P], wv_sb[:, kt, :],
                            start=(kt == 0), stop=(kt == NKT - 1),
                        )
                    nc.scalar.copy(vh_sb[:, kb, :], pv[:])
                return [(853, run)]

            def kv_dma(st):
                sl = slice(st * QB, (st + 1) * QB)
                kT_sb = sp.tile([P, NKT, QB], f16, tag="xin", bufs=8, name="kT_sb")
                nc.sync.dma_start(kT_sb[:], kT_r[:, :, sl])
                vT_sb = sp.tile([P, NKT, QB], f16, tag="xin", bufs=8, name="vT_sb")
                nc.sync.dma_start(vT_sb[:], vT_r[:, :, sl])
                return kT_sb, vT_sb

            def kv_pieces(st, kT_sb, vT_sb):
                ks = kchain_pieces(0, kT_sb, st) + kchain_pieces(1, kT_sb, st)
                vs = []
                for ss in range(QB // P):
                    vs += vchain_pieces(ss, vT_sb, st)
                return ks, vs

            def q_dma(qb):
                qsl = slice(qb * QB, (qb + 1) * QB)
                qT_sb = sp.tile([P, NKT, QB], f16, tag="xin", bufs=8, name="qT_sb")
                nc.sync.dma_start(qT_sb[:], qT_r[:, :, qsl])
                return qT_sb

            def qchain_pieces(ct, qT_sb, qhT_sb):
                cell = {}
                def half(h):
                    if h == 0:
                        cell["pq"] = psU.tile([P, QB], f32, tag="u", name="pq")
                    pq = cell["pq"]
                    for kt in range(h * NKT // 2, (h + 1) * NKT // 2):
                        nc.tensor.matmul(
                            pq[:], wq_sb[:, kt, ct * P:(ct + 1) * P], qT_sb[:, kt, :],
                            start=(kt == 0), stop=(kt == NKT - 1),
                        )
                    if h == 1:
                        nc.scalar.copy(qhT_sb[:, ct, :], pq[:])
                return [(853, lambda h=h: half(h)) for h in range(2)]

            def q_pieces(qT_sb):
                qhT_sb = sp.tile([P, NPAIR, QB], f16, tag="qh", bufs=3, name="qhT_sb")
                ps = qchain_pieces(0, qT_sb, qhT_sb) + qchain_pieces(1, qT_sb, qhT_sb)
                return qhT_sb, ps

            def o_proj_pieces(xts, qb, ssub):
                """One 128-row slice of the output projection, one piece per
                512-column chunk; the store DMA is issued from the Act queue
                right after the last PSUM drain so it never head-of-line
                blocks the SP DMA queue on a wait."""
                osb = sp.tile([P, D], f16, tag="osb", bufs=4, name="osb")
                rsl = slice(qb * QB + ssub * P, qb * QB + (ssub + 1) * P)
                def piece(et):
                    po = psU.tile([P, QB], f32, tag="u", name="po")
                    for ck in range(HD // P):
                        nc.tensor.matmul(
                            po[:],
                            xts[:, ck, ssub * P:(ssub + 1) * P],
                            wo_sb[:, ck, et * QB:(et + 1) * QB],
                            start=(ck == 0), stop=(ck == HD // P - 1),
                        )
                    esl = slice(et * QB, (et + 1) * QB)
                    nc.scalar.copy(osb[:, esl], po[:])
                    if et == D // QB - 1:
                        nc.scalar.dma_start(out[rsl, :], osb[:])
                return [(427, lambda et=et: piece(et)) for et in range(D // QB)]

            def o_proj_chain(xts, qb, ssub):
                for _, t in o_proj_pieces(xts, qb, ssub):
                    t()

            # ---- prologue DMAs: wk/kT0 interleaved halves so the first
            # k-chain piece starts as soon as possible
            kT0 = sp.tile([P, NKT, QB], f16, tag="xin", bufs=8, name="kT_sb")
            h1 = slice(0, NKT // 2)
            h2 = slice(NKT // 2, NKT)
            nc.sync.dma_start(wk_sb[:, h1, :], wk_r[:, h1, :])
            nc.sync.dma_start(kT0[:, h1, :], kT_r[:, h1, 0:QB])
            nc.sync.dma_start(wq_sb[:], wq_r[:])
            nc.sync.dma_start(wk_sb[:, h2, :], wk_r[:, h2, :])
            nc.sync.dma_start(kT0[:, h2, :], kT_r[:, h2, 0:QB])
            qT0 = q_dma(0)
            nc.sync.dma_start(wv_sb[:], wv_r[:])
            vT0 = sp.tile([P, NKT, QB], f16, tag="xin", bufs=8, name="vT_sb")
            nc.sync.dma_start(vT0[:], vT_r[:, :, 0:QB])
            A0 = sp.tile([P, AGRP, QB], f16, tag="A", bufs=4, name="A_sb")
            nc.sync.dma_start(A0[:], A_r[:, 0:AGRP, 0:QB])

            # prologue compute: k and q projections first so attention can
            # start; first halves of both k chains run back-to-back so the
            # second halves never outrun the second DMA chunk
            k0 = kchain_pieces(0, kT0, 0)
            k1 = kchain_pieces(1, kT0, 0)
            qhT_cur, qps = q_pieces(qT0)
            for _, t in [k0[0], k1[0], k0[1], k1[1]] + qps:
                t()
            kv_pre = kv_dma(1)
            nc.sync.dma_start(wo_sb[:], wo_r[:])

            fillers = []
            for ss in range(QB // P):
                fillers += vchain_pieces(ss, vT0, 0)

            pend_xts = None    # (xts tile, qb) awaiting output projection
            qhT_next = None
            qT_next = None
            A_next = None      # next qb's prefetched A tiles {g: tile}
            carry = []         # previous qb's unconsumed (kb, pt, j, xt, xts)

            def emit_pv(pt, kb, j, xt, xts):
                # p @ v: both heads of a pair run concurrently in the
                # left/right PE col-quadrants into one PSUM bank.
                nc.tensor.matmul(
                    xt[0:DK, j, :],
                    vh_sb[:, kb, (2 * j) * DK:(2 * j + 1) * DK],
                    pt[:, 0, :],
                    start=(kb == 0), stop=(kb == NKB - 1),
                    tile_position=(0, 0), skip_group_check=True,
                )
                nc.tensor.matmul(
                    xt[DK:P, j, :],
                    vh_sb[:, kb, (2 * j + 1) * DK:(2 * j + 2) * DK],
                    pt[:, 1, :],
                    start=(kb == 0), stop=(kb == NKB - 1),
                    tile_position=(0, DK), skip_group_check=True,
                )
                if kb == NKB - 1:
                    nc.scalar.copy(xts[:, j, :], xt[:, j, :])

            for qb in range(NQB):
                qsl = slice(qb * QB, (qb + 1) * QB)
                if qb == 0:
                    modes = MODES_QB0
                elif qb == NQB - 1:
                    modes = MODES_LAST
                else:
                    modes = MODES_STEADY
                xt = psX.tile([P, NPAIR, QB], f32, tag="xt", name="xt")  # 2 banks
                xts = sp.tile([P, NPAIR, QB], f32r, tag="xts", bufs=3, name="xts")
                pendA = []   # (pt tile, kb) for pair 0, consumed kb+3
                pendB = []   # (pt tile, kb) for pair 1, consumed kb+5

                def a_dma(g, qb_of):
                    t = sp.tile([P, AGRP, QB], f16, tag="A", bufs=4, name="A_sb")
                    nc.sync.dma_start(
                        t[:], A_r[:, g * AGRP:(g + 1) * AGRP,
                                  qb_of * QB:(qb_of + 1) * QB])
                    return t

                if qb == 0:
                    A_tiles = {0: A0, 1: a_dma(1, 0)}
                else:
                    A_tiles = A_next          # prefetched during previous qb
                A_next = {}

                for kb in range(NKB):
                    kbg, i = divmod(kb, AGRP)
                    if i == 0:
                        # A prefetch, two groups ahead (wraps into next qb)
                        tgt = kbg + 2
                        if tgt < NGRP:
                            A_tiles[tgt] = a_dma(tgt, qb)
                        elif qb < NQB - 1:
                            A_next[tgt - NGRP] = a_dma(tgt - NGRP, qb + 1)
                        if kbg == 1 and qb > 0 and pend_xts is not None:
                            xts_p, qb_p = pend_xts
                            for s in range(QB // P):
                                fillers += o_proj_pieces(xts_p, qb_p, s)
                            pend_xts = None
                        if kbg == 0:
                            if qb == 0:
                                ks, vs = kv_pieces(1, *kv_pre)
                                # k pieces jump the queue: the next group's
                                # scores need khT before pv needs vh
                                fillers = fillers[:2] + ks + fillers[2:] + vs
                                kv_pre = kv_dma(2)
                        if kbg == 1:
                            if qb == 0:
                                ks, vs = kv_pieces(2, *kv_pre)
                                fillers = fillers[:2] + ks + fillers[2:] + vs
                                kv_pre = kv_dma(3)
                            if qb < NQB - 1:
                                qT_next = q_dma(qb + 1)
                        # previous block's deferred p@v drains into the first
                        # key-blocks of this one (overlapping its mask latency)
                        if kbg == 2:
                            if qb == 0:
                                ks, vs = kv_pieces(3, *kv_pre)
                                fillers = fillers[:2] + ks + fillers[2:] + vs
                            if qb < NQB - 1:
                                qhT_next, qps = q_pieces(qT_next)
                                fillers += qps
                    for _ in range(4):
                        if not carry:
                            break
                        k0, pt, j, xt_o, xts_o = carry.pop(0)
                        emit_pv(pt, k0, j, xt_o, xts_o)
                    A_sb = A_tiles[kbg]
                    ksl = slice(kb * KBLK, (kb + 1) * KBLK)

                    # scores: the two heads of a pair run concurrently in the
                    # upper/lower PE row-quadrants, into the two banks of one
                    # PSUM pair-tile.  Pair 1 (the Act-bounced one) goes first
                    # so its bank recycles with the most PE-work cover.
                    scs = [None, None]
                    for j in (1, 0):
                        sc = psS.tile([P, 2, QB], f32, tag=("scA", "scB")[j],
                                      bufs=1, name=("scA", "scB")[j])
                        nc.tensor.matmul(
                            sc[:, 0, :], khT_sb[0:DK, j, ksl], qhT_cur[0:DK, j, :],
                            start=True, stop=True, tile_position=(0, 0),
                        )
                        nc.tensor.matmul(
                            sc[:, 1, :], khT_sb[DK:P, j, ksl], qhT_cur[DK:P, j, :],
                            start=True, stop=True, tile_position=(DK, 0),
                        )
                        scs[j] = sc

                        # issue the mask path for this pair immediately
                        a2 = A_sb[:, i, :].unsqueeze(1).broadcast_to([P, 2, QB])
                        if j == 1:
                            mode = modes[kb]
                            ptB = sp.tile([P, 2, QB], f16, tag="ptB", bufs=7,
                                          name="ptB")
                            if mode == "dve":
                                nc.vector.tensor_tensor(ptB[:], sc[:], a2,
                                                        mybir.AluOpType.mult)
                            else:
                                scb = sp.tile([P, 2, QB], f16, tag="scb", bufs=7,
                                              name="scb")
                                nc.scalar.copy(scb[:], sc[:])
                                eng = nc.gpsimd if mode == "pool" else nc.vector
                                eng.tensor_tensor(ptB[:], scb[:], a2,
                                                  mybir.AluOpType.mult)
                            pendB.append((ptB, kb))
                        else:
                            ptA = sp.tile([P, 2, QB], f16, tag="ptA", bufs=4,
                                          name="ptA")
                            nc.vector.tensor_tensor(ptA[:], sc[:], a2,
                                                    mybir.AluOpType.mult)
                            pendA.append((ptA, kb))

                    # software pipeline: consume pair-0 masks 3 kb behind,
                    # pair-1 masks 5 kb behind
                    if len(pendA) > 3:
                        pt, k0 = pendA.pop(0)
                        emit_pv(pt, k0, 0, xt, xts)
                    if len(pendB) > 5:
                        pt, k0 = pendB.pop(0)
                        emit_pv(pt, k0, 1, xt, xts)
                    budget = 1700 if qb == 0 else 420
                    spent = 0
                    while fillers and spent < budget:
                        cost, thunk = fillers.pop(0)
                        thunk()
                        spent += cost

                while fillers:
                    fillers.pop(0)[1]()
                carry = sorted(
                    [(k0, pt, 0, xt, xts) for pt, k0 in pendA] +
                    [(k0, pt, 1, xt, xts) for pt, k0 in pendB],
                    key=lambda c: (c[0], c[2]))
                pend_xts = (xts, qb)
                qhT_cur, qhT_next = qhT_next, None

            # tail: flush pair-0 pends first so its xT drain lands early,
            # then start the ck=0 half of the output projection (it reads only
            # pair 0) while pair-1 pends flush; finish with the ck=1 half
            xts_p, qb_p = pend_xts
            for k0, pt, j, xt_o, xts_o in sorted(carry, key=lambda c: (c[2], c[0])):
                emit_pv(pt, k0, j, xt_o, xts_o)
                if j == 0 and k0 == NKB - 1:
                    # pair-0 accumulator drained: start ck=0 matmuls
                    pos = {}
                    for ssub in range(QB // P):
                        for et in range(D // QB):
                            po = psU.tile([P, QB], f32, tag="u", bufs=4,
                                          name="po")
                            nc.tensor.matmul(
                                po[:], xts_p[:, 0, ssub * P:(ssub + 1) * P],
                                wo_sb[:, 0, et * QB:(et + 1) * QB],
                                start=True, stop=False,
                            )
                            pos[(ssub, et)] = po
            for ssub in range(QB // P):
                osb = sp.tile([P, D], f16, tag="osb", bufs=4, name="osb")
                rsl = slice(qb_p * QB + ssub * P, qb_p * QB + (ssub + 1) * P)
                for et in range(D // QB):
                    po = pos[(ssub, et)]
                    nc.tensor.matmul(
                        po[:], xts_p[:, 1, ssub * P:(ssub + 1) * P],
                        wo_sb[:, 1, et * QB:(et + 1) * QB],
                        start=False, stop=True,
                    )
                    esl = slice(et * QB, (et + 1) * QB)
                    if et == 0:
                        nc.vector.tensor_copy(osb[:, esl], po[:])
                    else:
                        nc.scalar.copy(osb[:, esl], po[:])
                nc.scalar.dma_start(out[rsl, :], osb[:])

    nc.compile()
    return nc


def _numpy_fallback(q, k, v, A, Wq, bq, Wk, bk, Wv, bv, Wo, bo):
    def proj(x, W, b):
        y = x @ W.T + b
        return y.reshape(B, S, H, DK).transpose(0, 2, 1, 3)

    qh, kh, vh = proj(q, Wq, bq), proj(k, Wk, bk), proj(v, Wv, bv)
    scores = np.einsum("bhqd,bhkd->bhqk", qh, kh) * np.float32(SCALE)
    p = scores * A.T
    x = np.einsum("bhqk,bhkd->bhqd", p, vh)
    x = x.transpose(0, 2, 1, 3).reshape(B, S, D)
    return (x @ Wo.T + bo).astype(np.float32)


def kernel(**inputs):
    q = np.asarray(inputs["q"], dtype=np.float32)
    k = np.asarray(inputs["k"], dtype=np.float32)
    v = np.asarray(inputs["v"], dtype=np.float32)
    A = np.asarray(inputs["A"], dtype=np.float32)
    Wq = np.asarray(inputs["Wq"], dtype=np.float32)
    Wk = np.asarray(inputs["Wk"], dtype=np.float32)
    Wv = np.asarray(inputs["Wv"], dtype=np.float32)
    Wo = np.asarray(inputs["Wo"], dtype=np.float32)
    bq, bk, bv, bo = (np.asarray(inputs[n], dtype=np.float32) for n in ("bq", "bk", "bv", "bo"))

    # The device kernel folds zero biases away (spec fills them with zeros);
    # fall back to a host reference in the (unused) nonzero-bias case.
    if any(np.any(b) for b in (bq, bk, bv)):
        return _numpy_fallback(q, k, v, A, Wq, bq, Wk, bk, Wv, bv, Wo, bo)

    global _CACHED
    if _CACHED is None:
        _CACHED = _build()
    nc = _CACHED

    Asc = np.ascontiguousarray((A * np.float32(SCALE)).astype(np.float16))
    in_maps = []
    for c in range(NCORES):
        b, g = divmod(c, GROUPS)
        hsl = slice(g * HD, (g + 1) * HD)
        in_maps.append({
            "qT": np.ascontiguousarray(q[b].T.astype(np.float16)),
            "kT": np.ascontiguousarray(k[b].T.astype(np.float16)),
            "vT": np.ascontiguousarray(v[b].T.astype(np.float16)),
            "Asc": Asc,
            "wq": np.ascontiguousarray(Wq[hsl].T.astype(np.float16)),
            "wk": np.ascontiguousarray(Wk[hsl].T.astype(np.float16)),
            "wv": np.ascontiguousarray(Wv[hsl].T.astype(np.float16)),
            "wo": np.ascontiguousarray(Wo[:, hsl].T),
        })

    res = bass_utils.run_bass_kernel_spmd(
        nc, in_maps, core_ids=list(range(NCORES)), trace=TRACE
    )
    global LAST_RESULTS
    LAST_RESULTS = res

    out = np.zeros((B, S, D), dtype=np.float32)
    for c in range(NCORES):
        out[c // GROUPS] += res.results[c]["out"].astype(np.float32)
    out += bo
    return out


if __name__ == "__main__":
    rng = np.random.default_rng(0)
    ins = {
        "q": rng.standard_normal((B, S, D), dtype=np.float32),
        "k": rng.standard_normal((B, S, D), dtype=np.float32),
        "v": rng.standard_normal((B, S, D), dtype=np.float32),
        "A": rng.random((S, S), dtype=np.float32),
        "Wq": rng.standard_normal((D, D), dtype=np.float32) / 32,
        "bq": np.zeros(D, np.float32),
        "Wk": rng.standard_normal((D, D), dtype=np.float32) / 32,
        "bk": np.zeros(D, np.float32),
        "Wv": rng.standard_normal((D, D), dtype=np.float32) / 32,
        "bv": np.zeros(D, np.float32),
        "Wo": rng.standard_normal((D, D), dtype=np.float32) / 32,
        "bo": np.zeros(D, np.float32),
    }
    got = kernel(**ins)
    ref = _numpy_fallback(**ins)
    err = np.abs(got - ref).max() / np.abs(ref).max()
    print("self-check relmax:", err)


# revision 25
# speedup vs baseline: 1.1337x; 1.0224x over previous
"""Trainium2 Bass kernel for nn_AttentionBlock (sparse_attention, no-softmax).

Computation (per batch b):
    qh = (q @ Wq^T) split into 16 heads of dk=64     [S, D] -> [H, S, DK]
    kh, vh likewise
    scores = (qh @ kh^T) / sqrt(DK)                  [H, S, S]
    p      = scores * A^T                            (elementwise structural mask)
    x      = p @ vh                                  [H, S, DK] -> [S, D]
    out    = x @ Wo^T + bo                           [S, D]

Sharding over 8 NeuronCores: data-parallel over batch (B=2) x tensor-parallel
over heads (16 heads -> 4 per core). Each core projects q/k/v for its 4 heads
(column-parallel), runs masked attention for them, and applies its 256-column
slice of the output projection (row-parallel), producing a full-shape partial
output. Host sums the 4 partials per batch.

Implementation notes:
- Activations ship pre-transposed ([D, S]); 1/sqrt(DK) is folded into the
  mask A on the host; whole data path in fp16 with fp32 PSUM accumulation.
- Heads are stored as pairs on the partition axis. Score matmuls of a pair
  run concurrently in the upper/lower PE row-quadrants into the TWO banks of
  one [128,2,512] PSUM tile; p@v matmuls run concurrently in left/right
  col-quadrants of one bank.
- The mask multiply processes a head-pair per instruction ([128,2,512], the
  A block shared across the pair via a stride-0 broadcast AP). Work is split
  three ways to fit under the PE time: DVE straight out of PSUM, an
  Activation PSUM->SBUF fp16 bounce feeding either GPSIMD or a 2x-mode DVE
  multiply (all-fp16-SBUF operands run at 2 elem/cycle on DVE).
- p@v consumption is software-pipelined 2 key-blocks behind the DVE-masked
  pair and 4 key-blocks behind the bounced pairs, hiding mask latency.
- Projection work for other blocks is interleaved into the attention loop;
  output is stored fp16 (partials summed on host in fp32).
"""

import numpy as np

import concourse.mybir as mybir
import concourse.tile as tile
from concourse import bacc, bass_utils

B, S, D, H = 2, 2048, 1024, 16
NCORES = 8
GROUPS = NCORES // B          # 4 head-groups
HPC = H // GROUPS             # 4 heads per core
DK = D // H                   # 64
HD = HPC * DK                 # 256 head-dim columns per core
NPAIR = HPC // 2              # 2 head pairs per core
SCALE = 1.0 / np.sqrt(DK)

P = 128                       # SBUF partitions
QB = 512                      # query block
NQB = S // QB                 # 4
KBLK = 128                    # key block
NKB = S // KBLK               # 16
NKT = D // P                  # 8 contraction chunks for projections
AGRP = 4                      # key-blocks per A-tile DMA
NGRP = NKB // AGRP            # 4 groups

f32 = mybir.dt.float32
f16 = mybir.dt.float16
f32r = mybir.dt.float32r

# per-kb mask path for the second head pair (first pair always DVE-from-PSUM).
# 'pool': Act bounce -> GPSIMD;  'dve2x': Act bounce -> DVE 2x;  'dve': DVE.
# The pool path has the longest latency, so it is confined to early key-blocks
# and its p@v consumption pended the deepest; late key-blocks use the fast
# paths so the end-of-block pipeline flush never waits on GPSIMD.
MODES_STEADY = ["pool", "dve2x", "pool", "pool",
                "pool", "dve2x", "pool", "pool",
                "pool", "pool", "pool", "pool",
                "dve2x", "pool", "dve2x", "dve"]
# qb0's PE span is projection-heavy (all of K/V): more slack for bounces
MODES_QB0 = ["pool", "pool", "pool", "pool",
             "pool", "pool", "dve2x", "pool",
             "pool", "pool", "pool", "pool",
             "pool", "dve2x", "dve2x", "dve"]
# last block: no pool in the final key-blocks so the tail flush+output
# projection never waits on GPSIMD latency
MODES_LAST = ["pool", "pool", "pool", "pool",
              "pool", "pool", "pool", "pool",
              "pool", "pool", "pool", "dve2x",
              "dve2x", "dve2x", "dve2x", "dve2x"]

_CACHED = None  # built module, reused across kernel() calls
TRACE = False         # set True (e.g. from test.py) to profile the NEFF
LAST_RESULTS = None   # BassKernelResults of the most recent run


def _build():
    nc = bacc.Bacc("TRN2", target_bir_lowering=False)

    qT = nc.dram_tensor("qT", [D, S], f16, kind="ExternalInput")
    kT = nc.dram_tensor("kT", [D, S], f16, kind="ExternalInput")
    vT = nc.dram_tensor("vT", [D, S], f16, kind="ExternalInput")
    Asc = nc.dram_tensor("Asc", [S, S], f16, kind="ExternalInput")
    wq = nc.dram_tensor("wq", [D, HD], f16, kind="ExternalInput")
    wk = nc.dram_tensor("wk", [D, HD], f16, kind="ExternalInput")
    wv = nc.dram_tensor("wv", [D, HD], f16, kind="ExternalInput")
    wo = nc.dram_tensor("wo", [HD, D], f32r, kind="ExternalInput")
    out = nc.dram_tensor("out", [S, D], f16, kind="ExternalOutput")

    qT_r = qT.rearrange("(kt p) s -> p kt s", p=P)
    kT_r = kT.rearrange("(kt p) s -> p kt s", p=P)
    vT_r = vT.rearrange("(kt p) s -> p kt s", p=P)
    wq_r = wq.rearrange("(kt p) c -> p kt c", p=P)
    wk_r = wk.rearrange("(kt p) c -> p kt c", p=P)
    wv_r = wv.rearrange("(kt p) c -> p kt c", p=P)
    wo_r = wo.rearrange("(ck p) e -> p ck e", p=P)
    A_r = Asc.rearrange("(kb p) q -> p kb q", p=P)

    with tile.TileContext(nc) as tc:
        with (
            tc.tile_pool(name="persist", bufs=1) as pp,
            tc.tile_pool(name="stream", bufs=2) as sp,
            tc.tile_pool(name="psU", bufs=2, space="PSUM") as psU,   # proj/oproj
            tc.tile_pool(name="psS", bufs=1, space="PSUM") as psS,   # score pairs
            tc.tile_pool(name="psX", bufs=1, space="PSUM") as psX,   # xT accum
        ):
            wk_sb = pp.tile([P, NKT, HD], f16, tag="wk")
            wv_sb = pp.tile([P, NKT, HD], f16, tag="wv")
            wq_sb = pp.tile([P, NKT, HD], f16, tag="wq")
            wo_sb = pp.tile([P, HD // P, D], f32r, tag="wo")

            # head-PAIR layout: pair j holds head 2j on partitions 0:64 and
            # head 2j+1 on 64:128
            khT_sb = pp.tile([P, NPAIR, S], f16, tag="khT")
            vh_sb = pp.tile([P, NKB, HD], f16, tag="vh")     # [ks%128, kb, c]

            # ---- interleavable projection work, split into ~850ns pieces ----
            # each piece is (cost_ns, thunk); PSUM tiles are allocated lazily
            # by the first piece of a chain and carried in a cell

            def kchain_pieces(ct, kT_sb, st):
                sl = slice(st * QB, (st + 1) * QB)
                cell = {}
                def half(h):
                    if h == 0:
                        cell["pk"] = psU.tile([P, QB], f32, tag="u", name="pk")
                    pk = cell["pk"]
                    for kt in range(h * NKT // 2, (h + 1) * NKT // 2):
                        nc.tensor.matmul(
                            pk[:], wk_sb[:, kt, ct * P:(ct + 1) * P], kT_sb[:, kt, :],
                            start=(kt == 0), stop=(kt == NKT - 1),
                        )
                    if h == 1:
                        nc.scalar.copy(khT_sb[:, ct, sl], pk[:])
                return [(853, lambda h=h: half(h)) for h in range(2)]

            def vchain_pieces(ssub, vT_sb, st):
                kb = st * (QB // P) + ssub
                def run():
                    pv = psU.tile([P, HD], f32, tag="u", name="pv")
                    for kt in range(NKT):
                        nc.tensor.matmul(
                            pv[:], vT_sb[:, kt, ssub * P:(ssub + 1) * P], wv_sb[:, kt, :],
                            start=(kt == 0), stop=(kt == NKT - 1),
                        )
                    nc.scalar.copy(vh_sb[:, kb, :], pv[:])
                return [(853, run)]

            def kv_dma(st):
                sl = slice(st * QB, (st + 1) * QB)
                kT_sb = sp.tile([P, NKT, QB], f16, tag="xin", bufs=8, name="kT_sb")
                nc.sync.dma_start(kT_sb[:], kT_r[:, :, sl])
                vT_sb = sp.tile([P, NKT, QB], f16, tag="xin", bufs=8, name="vT_sb")
                nc.sync.dma_start(vT_sb[:], vT_r[:, :, sl])
                return kT_sb, vT_sb

            def kv_pieces(st, kT_sb, vT_sb):
                ks = kchain_pieces(0, kT_sb, st) + kchain_pieces(1, kT_sb, st)
                vs = []
                for ss in range(QB // P):
                    vs += vchain_pieces(ss, vT_sb, st)
                return ks, vs

            def q_dma(qb):
                qsl = slice(qb * QB, (qb + 1) * QB)
                qT_sb = sp.tile([P, NKT, QB], f16, tag="xin", bufs=8, name="qT_sb")
                nc.sync.dma_start(qT_sb[:], qT_r[:, :, qsl])
                return qT_sb

            def qchain_pieces(ct, qT_sb, qhT_sb):
                cell = {}
                def half(h):
                    if h == 0:
                        cell["pq"] = psU.tile([P, QB], f32, tag="u", name="pq")
                    pq = cell["pq"]
                    for kt in range(h * NKT // 2, (h + 1) * NKT // 2):
                        nc.tensor.matmul(
                            pq[:], wq_sb[:, kt, ct * P:(ct + 1) * P], qT_sb[:, kt, :],
                            start=(kt == 0), stop=(kt == NKT - 1),
                        )
                    if h == 1:
                        nc.scalar.copy(qhT_sb[:, ct, :], pq[:])
                return [(853, lambda h=h: half(h)) for h in range(2)]

            def q_pieces(qT_sb):
                qhT_sb = sp.tile([P, NPAIR, QB], f16, tag="qh", bufs=3, name="qhT_sb")
                ps = qchain_pieces(0, qT_sb, qhT_sb) + qchain_pieces(1, qT_sb, qhT_sb)
                return qhT_sb, ps

            def o_proj_pieces(xts, qb, ssub):
                """One 128-row slice of the output projection: one piece per
                512-column chunk (matmuls + PSUM drain), then the store DMA as
                its own zero-cost piece — by the time it is dispatched the
                drains are done, so it never head-of-line blocks the SP
                queue."""
                osb = sp.tile([P, D], f16, tag="osb", bufs=4, name="osb")
                rsl = slice(qb * QB + ssub * P, qb * QB + (ssub + 1) * P)
                def piece(et):
                    po = psU.tile([P, QB], f32, tag="u", name="po")
                    for ck in range(HD // P):
                        nc.tensor.matmul(
                            po[:],
                            xts[:, ck, ssub * P:(ssub + 1) * P],
                            wo_sb[:, ck, et * QB:(et + 1) * QB],
                            start=(ck == 0), stop=(ck == HD // P - 1),
                        )
                    esl = slice(et * QB, (et + 1) * QB)
                    nc.scalar.copy(osb[:, esl], po[:])
                return [(427, lambda et=et: piece(et)) for et in range(D // QB)] + \
                       [(0, lambda: nc.sync.dma_start(out[rsl, :], osb[:]))]

            def o_proj_chain(xts, qb, ssub):
                for _, t in o_proj_pieces(xts, qb, ssub):
                    t()

            # ---- prologue DMAs: wk/kT0 interleaved halves so the first
            # k-chain piece starts as soon as possible
            kT0 = sp.tile([P, NKT, QB], f16, tag="xin", bufs=8, name="kT_sb")
            h1 = slice(0, NKT // 2)
            h2 = slice(NKT // 2, NKT)
            nc.sync.dma_start(wk_sb[:, h1, :], wk_r[:, h1, :])
            nc.sync.dma_start(kT0[:, h1, :], kT_r[:, h1, 0:QB])
            nc.sync.dma_start(wq_sb[:], wq_r[:])
            nc.sync.dma_start(wk_sb[:, h2, :], wk_r[:, h2, :])
            nc.sync.dma_start(kT0[:, h2, :], kT_r[:, h2, 0:QB])
            qT0 = q_dma(0)
            nc.sync.dma_start(wv_sb[:], wv_r[:])
            vT0 = sp.tile([P, NKT, QB], f16, tag="xin", bufs=8, name="vT_sb")
            nc.sync.dma_start(vT0[:], vT_r[:, :, 0:QB])
            A0 = sp.tile([P, AGRP, QB], f16, tag="A", bufs=4, name="A_sb")
            nc.sync.dma_start(A0[:], A_r[:, 0:AGRP, 0:QB])

            # prologue compute: k and q projections first so attention can
            # start; first halves of both k chains run back-to-back so the
            # second halves never outrun the second DMA chunk
            k0 = kchain_pieces(0, kT0, 0)
            k1 = kchain_pieces(1, kT0, 0)
            qhT_cur, qps = q_pieces(qT0)
            for _, t in [k0[0], k1[0], k0[1], k1[1]] + qps:
                t()
            kv_pre = kv_dma(1)

            fillers = []
            for ss in range(QB // P):
                fillers += vchain_pieces(ss, vT0, 0)

            pend_xts = None    # (xts tile, qb) awaiting output projection
            qhT_next = None
            qT_next = None
            A_next = None      # next qb's prefetched A tiles {g: tile}
            carry = []         # previous qb's unconsumed (kb, pt, j, xt, xts)

            def emit_pv(pt, kb, j, xt, xts):
                # p @ v: both heads of a pair run concurrently in the
                # left/right PE col-quadrants into one PSUM bank.
                nc.tensor.matmul(
                    xt[0:DK, j, :],
                    vh_sb[:, kb, (2 * j) * DK:(2 * j + 1) * DK],
                    pt[:, 0, :],
                    start=(kb == 0), stop=(kb == NKB - 1),
                    tile_position=(0, 0), skip_group_check=True,
                )
                nc.tensor.matmul(
                    xt[DK:P, j, :],
                    vh_sb[:, kb, (2 * j + 1) * DK:(2 * j + 2) * DK],
                    pt[:, 1, :],
                    start=(kb == 0), stop=(kb == NKB - 1),
                    tile_position=(0, DK), skip_group_check=True,
                )
                if kb == NKB - 1:
                    nc.scalar.copy(xts[:, j, :], xt[:, j, :])

            for qb in range(NQB):
                qsl = slice(qb * QB, (qb + 1) * QB)
                if qb == 0:
                    modes = MODES_QB0
                elif qb == NQB - 1:
                    modes = MODES_LAST
                else:
                    modes = MODES_STEADY
                xt = psX.tile([P, NPAIR, QB], f32, tag="xt", name="xt")  # 2 banks
                xts = sp.tile([P, NPAIR, QB], f32r, tag="xts", bufs=3, name="xts")
                pendA = []   # (pt tile, kb) for pair 0, consumed kb+3
                pendB = []   # (pt tile, kb) for pair 1, consumed kb+5

                def a_dma(g, qb_of):
                    t = sp.tile([P, AGRP, QB], f16, tag="A", bufs=4, name="A_sb")
                    nc.sync.dma_start(
                        t[:], A_r[:, g * AGRP:(g + 1) * AGRP,
                                  qb_of * QB:(qb_of + 1) * QB])
                    return t

                if qb == 0:
                    A_tiles = {0: A0, 1: a_dma(1, 0)}
                else:
                    A_tiles = A_next          # prefetched during previous qb
                A_next = {}

                for kb in range(NKB):
                    kbg, i = divmod(kb, AGRP)
                    if i == 0:
                        # A prefetch, two groups ahead (wraps into next qb)
                        tgt = kbg + 2
                        if tgt < NGRP:
                            A_tiles[tgt] = a_dma(tgt, qb)
                        elif qb < NQB - 1:
                            A_next[tgt - NGRP] = a_dma(tgt - NGRP, qb + 1)
                        if kbg == 1 and qb > 0 and pend_xts is not None:
                            xts_p, qb_p = pend_xts
                            for s in range(QB // P):
                                fillers += o_proj_pieces(xts_p, qb_p, s)
                            pend_xts = None
                        if kbg == 0:
                            if qb == 0:
                                ks, vs = kv_pieces(1, *kv_pre)
                                # k pieces jump the queue: the next group's
                                # scores need khT before pv needs vh
                                fillers = fillers[:2] + ks + fillers[2:] + vs
                                kv_pre = kv_dma(2)
                        if kbg == 1:
                            if qb == 0:
                                ks, vs = kv_pieces(2, *kv_pre)
                                fillers = fillers[:2] + ks + fillers[2:] + vs
                                kv_pre = kv_dma(3)
                            if qb < NQB - 1:
                                qT_next = q_dma(qb + 1)
                        # previous block's deferred p@v drains into the first
                        # key-blocks of this one (overlapping its mask latency)
                        if kbg == 2:
                            if qb == 0:
                                ks, vs = kv_pieces(3, *kv_pre)
                                fillers = fillers[:2] + ks + fillers[2:] + vs
                                nc.sync.dma_start(wo_sb[:], wo_r[:])
                            if qb < NQB - 1:
                                qhT_next, qps = q_pieces(qT_next)
                                fillers += qps
                    for _ in range(4):
                        if not carry:
                            break
                        k0, pt, j, xt_o, xts_o = carry.pop(0)
                        emit_pv(pt, k0, j, xt_o, xts_o)
                    A_sb = A_tiles[kbg]
                    ksl = slice(kb * KBLK, (kb + 1) * KBLK)

                    # scores: the two heads of a pair run concurrently in the
                    # upper/lower PE row-quadrants, into the two banks of one
                    # PSUM pair-tile.  Pair 1 (the Act-bounced one) goes first
                    # so its bank recycles with the most PE-work cover.
                    a2 = A_sb[:, i, :].unsqueeze(1).broadcast_to([P, 2, QB])
                    # pair 1 first: its Act bounce recycles the bank fastest
                    sc = psS.tile([P, 2, QB], f32, tag="scB", bufs=1, name="scB")
                    nc.tensor.matmul(
                        sc[:, 0, :], khT_sb[0:DK, 1, ksl], qhT_cur[0:DK, 1, :],
                        start=True, stop=True, tile_position=(0, 0),
                    )
                    nc.tensor.matmul(
                        sc[:, 1, :], khT_sb[DK:P, 1, ksl], qhT_cur[DK:P, 1, :],
                        start=True, stop=True, tile_position=(DK, 0),
                    )
                    mode = modes[kb]
                    ptB = sp.tile([P, 2, QB], f16, tag="ptB", bufs=8,
                                  name="ptB")
                    if mode == "dve":
                        nc.vector.tensor_tensor(ptB[:], sc[:], a2,
                                                mybir.AluOpType.mult)
                    else:
                        scb = sp.tile([P, 2, QB], f16, tag="scb", bufs=8,
                                      name="scb")
                        nc.scalar.copy(scb[:], sc[:])
                        eng = nc.gpsimd if mode == "pool" else nc.vector
                        eng.tensor_tensor(ptB[:], scb[:], a2,
                                          mybir.AluOpType.mult)
                    pendB.append((ptB, kb))
                    # pair 0: two single-bank tiles masked by two single DVE
                    # ops, issue order alternating by kb parity so each bank
                    # sees a short recycle loop every other block
                    ptA = sp.tile([P, 2, QB], f16, tag="ptA", bufs=6,
                                  name="ptA")
                    halves = []
                    for h in range(2):
                        scs1 = psS.tile([P, QB], f32, tag=("scA0", "scA1")[h],
                                        bufs=1, name=("scA0", "scA1")[h])
                        nc.tensor.matmul(
                            scs1[:], khT_sb[h * DK:(h + 1) * DK, 0, ksl],
                            qhT_cur[h * DK:(h + 1) * DK, 0, :],
                            start=True, stop=True, tile_position=(h * DK, 0),
                        )
                        halves.append(scs1)
                    order = (0, 1) if kb % 2 == 0 else (1, 0)
                    for h in order:
                        nc.vector.tensor_tensor(ptA[:, h, :], halves[h][:],
                                                A_sb[:, i, :],
                                                mybir.AluOpType.mult)
                    pendA.append((ptA, kb))

                    # software pipeline: consume pair-0 masks 3 kb behind,
                    # pair-1 masks 5 kb behind
                    if len(pendA) > 4:
                        pt, k0 = pendA.pop(0)
                        emit_pv(pt, k0, 0, xt, xts)
                    if len(pendB) > 6:
                        pt, k0 = pendB.pop(0)
                        emit_pv(pt, k0, 1, xt, xts)
                    budget = 1700 if qb == 0 else 420
                    spent = 0
                    while fillers and spent < budget:
                        cost, thunk = fillers.pop(0)
                        thunk()
                        spent += cost

                while fillers:
                    fillers.pop(0)[1]()
                carry = sorted(
                    [(k0, pt, 0, xt, xts) for pt, k0 in pendA] +
                    [(k0, pt, 1, xt, xts) for pt, k0 in pendB],
                    key=lambda c: (c[0], c[2]))
                pend_xts = (xts, qb)
                qhT_cur, qhT_next = qhT_next, None

            # tail: flush the last deferred p@v, then the output projection
            # cycling PSUM through the now-free score banks, drains alternating
            # between DVE and Act, and ONE merged store DMA at the end
            xts_p, qb_p = pend_xts
            for k0, pt, j, xt_o, xts_o in carry:
                emit_pv(pt, k0, j, xt_o, xts_o)
            osb_t = sp.tile([P, QB // P, D], f16, tag="osbt", bufs=1,
                            name="osb_t")
            po_slots = [lambda: psU.tile([P, QB], f32, tag="u", name="po"),
                        lambda: psS.tile([P, QB], f32, tag="scA0", bufs=1,
                                         name="po_a"),
                        lambda: psU.tile([P, QB], f32, tag="u", name="po"),
                        lambda: psS.tile([P, QB], f32, tag="scA1", bufs=1,
                                         name="po_b")]
            n = 0
            for ssub in range(QB // P):
                for et in range(D // QB):
                    po = po_slots[n % 4]()
                    n += 1
                    for ck in range(HD // P):
                        nc.tensor.matmul(
                            po[:], xts_p[:, ck, ssub * P:(ssub + 1) * P],
                            wo_sb[:, ck, et * QB:(et + 1) * QB],
                            start=(ck == 0), stop=(ck == HD // P - 1),
                        )
                    esl = slice(et * QB, (et + 1) * QB)
                    if et == 0:
                        nc.vector.tensor_copy(osb_t[:, ssub, esl], po[:])
                    else:
                        nc.scalar.copy(osb_t[:, ssub, esl], po[:])
            out_r = out.rearrange("(qq p) d -> p qq d", p=P)
            nc.sync.dma_start(
                out_r[:, qb_p * (QB // P):(qb_p + 1) * (QB // P), :], osb_t[:])

    nc.compile()
    return nc


def _numpy_fallback(q, k, v, A, Wq, bq, Wk, bk, Wv, bv, Wo, bo):
    def proj(x, W, b):
        y = x @ W.T + b
        return y.reshape(B, S, H, DK).transpose(0, 2, 1, 3)

    qh, kh, vh = proj(q, Wq, bq), proj(k, Wk, bk), proj(v, Wv, bv)
    scores = np.einsum("bhqd,bhkd->bhqk", qh, kh) * np.float32(SCALE)
    p = scores * A.T
    x = np.einsum("bhqk,bhkd->bhqd", p, vh)
    x = x.transpose(0, 2, 1, 3).reshape(B, S, D)
    return (x @ Wo.T + bo).astype(np.float32)


def kernel(**inputs):
    q = np.asarray(inputs["q"], dtype=np.float32)
    k = np.asarray(inputs["k"], dtype=np.float32)
    v = np.asarray(inputs["v"], dtype=np.float32)
    A = np.asarray(inputs["A"], dtype=np.float32)
    Wq = np.asarray(inputs["Wq"], dtype=np.float32)
    Wk = np.asarray(inputs["Wk"], dtype=np.float32)
    Wv = np.asarray(inputs["Wv"], dtype=np.float32)
    Wo = np.asarray(inputs["Wo"], dtype=np.float32)
    bq, bk, bv, bo = (np.asarray(inputs[n], dtype=np.float32) for n in ("bq", "bk", "bv", "bo"))

    # The device kernel folds zero biases away (spec fills them with zeros);
    # fall back to a host reference in the (unused) nonzero-bias case.
    if any(np.any(b) for b in (bq, bk, bv)):
        return _numpy_fallback(q, k, v, A, Wq, bq, Wk, bk, Wv, bv, Wo, bo)

    global _CACHED
    if _CACHED is None:
        _CACHED = _build()
    nc = _CACHED

    Asc = np.ascontiguousarray((A * np.float32(SCALE)).astype(np.float16))
    in_maps = []
    for c in range(NCORES):
        b, g = divmod(c, GROUPS)
        hsl = slice(g * HD, (g + 1) * HD)
        in_maps.append({
            "qT": np.ascontiguousarray(q[b].T.astype(np.float16)),
            "kT": np.ascontiguousarray(k[b].T.astype(np.float16)),
            "vT": np.ascontiguousarray(v[b].T.astype(np.float16)),
            "Asc": Asc,
            "wq": np.ascontiguousarray(Wq[hsl].T.astype(np.float16)),
            "wk": np.ascontiguousarray(Wk[hsl].T.astype(np.float16)),
            "wv": np.ascontiguousarray(Wv[hsl].T.astype(np.float16)),
            "wo": np.ascontiguousarray(Wo[:, hsl].T),
        })

    res = bass_utils.run_bass_kernel_spmd(
        nc, in_maps, core_ids=list(range(NCORES)), trace=TRACE
    )
    global LAST_RESULTS
    LAST_RESULTS = res

    out = np.zeros((B, S, D), dtype=np.float32)
    for c in range(NCORES):
        out[c // GROUPS] += res.results[c]["out"].astype(np.float32)
    out += bo
    return out


if __name__ == "__main__":
    rng = np.random.default_rng(0)
    ins = {
        "q": rng.standard_normal((B, S, D), dtype=np.float32),
        "k": rng.standard_normal((B, S, D), dtype=np.float32),
        "v": rng.standard_normal((B, S, D), dtype=np.float32),
        "A": rng.random((S, S), dtype=np.float32),
        "Wq": rng.standard_normal((D, D), dtype=np.float32) / 32,
        "bq": np.zeros(D, np.float32),
        "Wk": rng.standard_normal((D, D), dtype=np.float32) / 32,
        "bk": np.zeros(D, np.float32),
        "Wv": rng.standard_normal((D, D), dtype=np.float32) / 32,
        "bv": np.zeros(D, np.float32),
        "Wo": rng.standard_normal((D, D), dtype=np.float32) / 32,
        "bo": np.zeros(D, np.float32),
    }
    got = kernel(**ins)
    ref = _numpy_fallback(**ins)
    err = np.abs(got - ref).max() / np.abs(ref).max()
    print("self-check relmax:", err)


# revision 28
# speedup vs baseline: 1.1589x; 1.0223x over previous
"""Trainium2 Bass kernel for nn_AttentionBlock (sparse_attention, no-softmax).

Computation (per batch b):
    qh = (q @ Wq^T) split into 16 heads of dk=64     [S, D] -> [H, S, DK]
    kh, vh likewise
    scores = (qh @ kh^T) / sqrt(DK)                  [H, S, S]
    p      = scores * A^T                            (elementwise structural mask)
    x      = p @ vh                                  [H, S, DK] -> [S, D]
    out    = x @ Wo^T + bo                           [S, D]

Sharding over 8 NeuronCores: data-parallel over batch (B=2) x tensor-parallel
over heads (16 heads -> 4 per core). Each core projects q/k/v for its 4 heads
(column-parallel), runs masked attention for them, and applies its 256-column
slice of the output projection (row-parallel), producing a full-shape partial
output. Host sums the 4 partials per batch.

Implementation notes:
- Activations ship pre-transposed ([D, S]); 1/sqrt(DK) is folded into the
  mask A on the host; whole data path in fp16 with fp32 PSUM accumulation.
- Heads are stored as pairs on the partition axis. Score matmuls of a pair
  run concurrently in the upper/lower PE row-quadrants into the TWO banks of
  one [128,2,512] PSUM tile; p@v matmuls run concurrently in left/right
  col-quadrants of one bank.
- The mask multiply processes a head-pair per instruction ([128,2,512], the
  A block shared across the pair via a stride-0 broadcast AP). Work is split
  three ways to fit under the PE time: DVE straight out of PSUM, an
  Activation PSUM->SBUF fp16 bounce feeding either GPSIMD or a 2x-mode DVE
  multiply (all-fp16-SBUF operands run at 2 elem/cycle on DVE).
- p@v consumption is software-pipelined 2 key-blocks behind the DVE-masked
  pair and 4 key-blocks behind the bounced pairs, hiding mask latency.
- Projection work for other blocks is interleaved into the attention loop;
  output is stored fp16 (partials summed on host in fp32).
"""

import numpy as np

import concourse.mybir as mybir
import concourse.tile as tile
from concourse import bacc, bass_utils

B, S, D, H = 2, 2048, 1024, 16
NCORES = 8
GROUPS = NCORES // B          # 4 head-groups
HPC = H // GROUPS             # 4 heads per core
DK = D // H                   # 64
HD = HPC * DK                 # 256 head-dim columns per core
NPAIR = HPC // 2              # 2 head pairs per core
SCALE = 1.0 / np.sqrt(DK)

P = 128                       # SBUF partitions
QB = 512                      # query block
NQB = S // QB                 # 4
KBLK = 128                    # key block
NKB = S // KBLK               # 16
NKT = D // P                  # 8 contraction chunks for projections
AGRP = 4                      # key-blocks per A-tile DMA
NGRP = NKB // AGRP            # 4 groups

f32 = mybir.dt.float32
f16 = mybir.dt.float16
f32r = mybir.dt.float32r

# per-kb mask path for the second head pair (first pair always DVE-from-PSUM).
# 'pool': Act bounce -> GPSIMD;  'dve2x': Act bounce -> DVE 2x;  'dve': DVE.
# The pool path has the longest latency, so it is confined to early key-blocks
# and its p@v consumption pended the deepest; late key-blocks use the fast
# paths so the end-of-block pipeline flush never waits on GPSIMD.
MODES_STEADY = ["pool", "dve2x", "pool", "pool",
                "pool", "dve2x", "pool", "pool",
                "pool", "pool", "pool", "pool",
                "dve2x", "pool", "dve2x", "dve"]
# qb0's PE span is projection-heavy (all of K/V): more slack for bounces
MODES_QB0 = ["pool", "pool", "pool", "pool",
             "pool", "pool", "dve2x", "pool",
             "pool", "pool", "pool", "pool",
             "pool", "dve2x", "dve2x", "dve"]
# last block: no pool in the final key-blocks so the tail flush+output
# projection never waits on GPSIMD latency
MODES_LAST = ["pool", "pool", "pool", "pool",
              "pool", "pool", "pool", "pool",
              "pool", "pool", "pool", "dve2x",
              "dve2x", "dve2x", "dve2x", "dve2x"]

_CACHED = None  # built module, reused across kernel() calls
TRACE = False         # set True (e.g. from test.py) to profile the NEFF
LAST_RESULTS = None   # BassKernelResults of the most recent run


def _build():
    nc = bacc.Bacc("TRN2", target_bir_lowering=False)

    qT = nc.dram_tensor("qT", [D, S], f16, kind="ExternalInput")
    kT = nc.dram_tensor("kT", [D, S], f16, kind="ExternalInput")
    vT = nc.dram_tensor("vT", [D, S], f16, kind="ExternalInput")
    Asc = nc.dram_tensor("Asc", [S, S], f16, kind="ExternalInput")
    wq = nc.dram_tensor("wq", [D, HD], f16, kind="ExternalInput")
    wk = nc.dram_tensor("wk", [D, HD], f16, kind="ExternalInput")
    wv = nc.dram_tensor("wv", [D, HD], f16, kind="ExternalInput")
    wo = nc.dram_tensor("wo", [HD, D], f32r, kind="ExternalInput")
    out = nc.dram_tensor("out", [S, D], f16, kind="ExternalOutput")

    qT_r = qT.rearrange("(kt p) s -> p kt s", p=P)
    kT_r = kT.rearrange("(kt p) s -> p kt s", p=P)
    vT_r = vT.rearrange("(kt p) s -> p kt s", p=P)
    wq_r = wq.rearrange("(kt p) c -> p kt c", p=P)
    wk_r = wk.rearrange("(kt p) c -> p kt c", p=P)
    wv_r = wv.rearrange("(kt p) c -> p kt c", p=P)
    wo_r = wo.rearrange("(ck p) e -> p ck e", p=P)
    A_r = Asc.rearrange("(kb p) q -> p kb q", p=P)

    with tile.TileContext(nc) as tc:
        with (
            tc.tile_pool(name="persist", bufs=1) as pp,
            tc.tile_pool(name="stream", bufs=2) as sp,
            tc.tile_pool(name="psU", bufs=2, space="PSUM") as psU,   # proj/oproj
            tc.tile_pool(name="psS", bufs=1, space="PSUM") as psS,   # score pairs
            tc.tile_pool(name="psX", bufs=1, space="PSUM") as psX,   # xT accum
        ):
            wk_sb = pp.tile([P, NKT, HD], f16, tag="wk")
            wv_sb = pp.tile([P, NKT, HD], f16, tag="wv")
            wq_sb = pp.tile([P, NKT, HD], f16, tag="wq")
            wo_sb = pp.tile([P, HD // P, D], f32r, tag="wo")

            # head-PAIR layout: pair j holds head 2j on partitions 0:64 and
            # head 2j+1 on 64:128
            khT_sb = pp.tile([P, NPAIR, S], f16, tag="khT")
            vh_sb = pp.tile([P, NKB, HD], f16, tag="vh")     # [ks%128, kb, c]

            # ---- interleavable projection work, split into ~850ns pieces ----
            # each piece is (cost_ns, thunk); PSUM tiles are allocated lazily
            # by the first piece of a chain and carried in a cell

            def kchain_pieces(ct, kT_sb, st):
                sl = slice(st * QB, (st + 1) * QB)
                cell = {}
                def half(h):
                    if h == 0:
                        cell["pk"] = psU.tile([P, QB], f32, tag="u", name="pk")
                    pk = cell["pk"]
                    for kt in range(h * NKT // 2, (h + 1) * NKT // 2):
                        nc.tensor.matmul(
                            pk[:], wk_sb[:, kt, ct * P:(ct + 1) * P], kT_sb[:, kt, :],
                            start=(kt == 0), stop=(kt == NKT - 1),
                        )
                    if h == 1:
                        nc.scalar.copy(khT_sb[:, ct, sl], pk[:])
                return [(853, lambda h=h: half(h)) for h in range(2)]

            def vchain_pieces(ssub, vT_sb, st):
                kb = st * (QB // P) + ssub
                def run():
                    pv = psU.tile([P, HD], f32, tag="u", name="pv")
                    for kt in range(NKT):
                        nc.tensor.matmul(
                            pv[:], vT_sb[:, kt, ssub * P:(ssub + 1) * P], wv_sb[:, kt, :],
                            start=(kt == 0), stop=(kt == NKT - 1),
                        )
                    nc.scalar.copy(vh_sb[:, kb, :], pv[:])
                return [(853, run)]

            def kv_dma(st):
                sl = slice(st * QB, (st + 1) * QB)
                kT_sb = sp.tile([P, NKT, QB], f16, tag="xin", bufs=8, name="kT_sb")
                nc.sync.dma_start(kT_sb[:], kT_r[:, :, sl])
                vT_sb = sp.tile([P, NKT, QB], f16, tag="xin", bufs=8, name="vT_sb")
                nc.sync.dma_start(vT_sb[:], vT_r[:, :, sl])
                return kT_sb, vT_sb

            def kv_pieces(st, kT_sb, vT_sb):
                ks = kchain_pieces(0, kT_sb, st) + kchain_pieces(1, kT_sb, st)
                vs = []
                for ss in range(QB // P):
                    vs += vchain_pieces(ss, vT_sb, st)
                return ks, vs

            def q_dma(qb):
                qsl = slice(qb * QB, (qb + 1) * QB)
                qT_sb = sp.tile([P, NKT, QB], f16, tag="xin", bufs=8, name="qT_sb")
                nc.sync.dma_start(qT_sb[:], qT_r[:, :, qsl])
                return qT_sb

            def qchain_pieces(ct, qT_sb, qhT_sb):
                cell = {}
                def half(h):
                    if h == 0:
                        cell["pq"] = psU.tile([P, QB], f32, tag="u", name="pq")
                    pq = cell["pq"]
                    for kt in range(h * NKT // 2, (h + 1) * NKT // 2):
                        nc.tensor.matmul(
                            pq[:], wq_sb[:, kt, ct * P:(ct + 1) * P], qT_sb[:, kt, :],
                            start=(kt == 0), stop=(kt == NKT - 1),
                        )
                    if h == 1:
                        nc.scalar.copy(qhT_sb[:, ct, :], pq[:])
                return [(853, lambda h=h: half(h)) for h in range(2)]

            def q_pieces(qT_sb):
                qhT_sb = sp.tile([P, NPAIR, QB], f16, tag="qh", bufs=3, name="qhT_sb")
                ps = qchain_pieces(0, qT_sb, qhT_sb) + qchain_pieces(1, qT_sb, qhT_sb)
                return qhT_sb, ps

            def o_proj_pieces(xts, qb, ssub):
                """One 128-row slice of the output projection: one piece per
                512-column chunk (matmuls + PSUM drain), then the store DMA as
                its own zero-cost piece — by the time it is dispatched the
                drains are done, so it never head-of-line blocks the SP
                queue."""
                osb = sp.tile([P, D], f16, tag="osb", bufs=4, name="osb")
                rsl = slice(qb * QB + ssub * P, qb * QB + (ssub + 1) * P)
                def piece(et):
                    po = psU.tile([P, QB], f32, tag="u", name="po")
                    for ck in range(HD // P):
                        nc.tensor.matmul(
                            po[:],
                            xts[:, ck, ssub * P:(ssub + 1) * P],
                            wo_sb[:, ck, et * QB:(et + 1) * QB],
                            start=(ck == 0), stop=(ck == HD // P - 1),
                        )
                    esl = slice(et * QB, (et + 1) * QB)
                    nc.scalar.copy(osb[:, esl], po[:])
                return [(427, lambda et=et: piece(et)) for et in range(D // QB)] + \
                       [(0, lambda: nc.sync.dma_start(out[rsl, :], osb[:]))]

            def o_proj_chain(xts, qb, ssub):
                for _, t in o_proj_pieces(xts, qb, ssub):
                    t()

            # ---- prologue DMAs: wk/kT0 interleaved halves so the first
            # k-chain piece starts as soon as possible
            kT0 = sp.tile([P, NKT, QB], f16, tag="xin", bufs=8, name="kT_sb")
            h1 = slice(0, NKT // 2)
            h2 = slice(NKT // 2, NKT)
            nc.sync.dma_start(wk_sb[:, h1, :], wk_r[:, h1, :])
            nc.sync.dma_start(kT0[:, 0:2, :], kT_r[:, 0:2, 0:QB])
            nc.sync.dma_start(kT0[:, 2:4, :], kT_r[:, 2:4, 0:QB])
            nc.sync.dma_start(wq_sb[:], wq_r[:])
            nc.sync.dma_start(wk_sb[:, h2, :], wk_r[:, h2, :])
            nc.sync.dma_start(kT0[:, h2, :], kT_r[:, h2, 0:QB])
            qT0 = q_dma(0)
            nc.sync.dma_start(wv_sb[:], wv_r[:])
            vT0 = sp.tile([P, NKT, QB], f16, tag="xin", bufs=8, name="vT_sb")
            nc.sync.dma_start(vT0[:], vT_r[:, :, 0:QB])
            A0 = sp.tile([P, AGRP, QB], f16, tag="A", bufs=4, name="A_sb")
            nc.sync.dma_start(A0[:], A_r[:, 0:AGRP, 0:QB])

            # prologue compute: k and q projections first so attention can
            # start; first halves of both k chains run back-to-back so the
            # second halves never outrun the second DMA chunk
            k0 = kchain_pieces(0, kT0, 0)
            k1 = kchain_pieces(1, kT0, 0)
            qhT_cur, qps = q_pieces(qT0)
            for _, t in [k0[0], k1[0], k0[1], k1[1]] + qps:
                t()
            kv_pre = kv_dma(1)

            fillers = []
            for ss in range(QB // P):
                fillers += vchain_pieces(ss, vT0, 0)

            pend_xts = None    # (xts tile, qb) awaiting output projection
            qhT_next = None
            qT_next = None
            A_next = None      # next qb's prefetched A tiles {g: tile}
            carry = []         # previous qb's unconsumed (kb, pt, j, xt, xts)

            def emit_pv(pt, kb, j, xt, xts):
                # p @ v: both heads of a pair run concurrently in the
                # left/right PE col-quadrants into one PSUM bank.
                nc.tensor.matmul(
                    xt[0:DK, j, :],
                    vh_sb[:, kb, (2 * j) * DK:(2 * j + 1) * DK],
                    pt[:, 0, :],
                    start=(kb == 0), stop=(kb == NKB - 1),
                    tile_position=(0, 0), skip_group_check=True,
                )
                nc.tensor.matmul(
                    xt[DK:P, j, :],
                    vh_sb[:, kb, (2 * j + 1) * DK:(2 * j + 2) * DK],
                    pt[:, 1, :],
                    start=(kb == 0), stop=(kb == NKB - 1),
                    tile_position=(0, DK), skip_group_check=True,
                )
                if kb == NKB - 1:
                    nc.scalar.copy(xts[:, j, :], xt[:, j, :])

            for qb in range(NQB):
                qsl = slice(qb * QB, (qb + 1) * QB)
                if qb == 0:
                    modes = MODES_QB0
                elif qb == NQB - 1:
                    modes = MODES_LAST
                else:
                    modes = MODES_STEADY
                xt = psX.tile([P, NPAIR, QB], f32, tag="xt", name="xt")  # 2 banks
                xts = sp.tile([P, NPAIR, QB], f32r, tag="xts", bufs=3, name="xts")
                pendA = []   # (pt tile, kb) for pair 0, consumed kb+3
                pendB = []   # (pt tile, kb) for pair 1, consumed kb+5

                def a_dma(g, qb_of):
                    t = sp.tile([P, AGRP, QB], f16, tag="A", bufs=4, name="A_sb")
                    nc.sync.dma_start(
                        t[:], A_r[:, g * AGRP:(g + 1) * AGRP,
                                  qb_of * QB:(qb_of + 1) * QB])
                    return t

                if qb == 0:
                    A_tiles = {0: A0, 1: a_dma(1, 0)}
                else:
                    A_tiles = A_next          # prefetched during previous qb
                A_next = {}

                for kb in range(NKB):
                    kbg, i = divmod(kb, AGRP)
                    if i == 0:
                        # A prefetch, two groups ahead (wraps into next qb)
                        tgt = kbg + 2
                        if tgt < NGRP:
                            A_tiles[tgt] = a_dma(tgt, qb)
                        elif qb < NQB - 1:
                            A_next[tgt - NGRP] = a_dma(tgt - NGRP, qb + 1)
                        if kbg == 1 and qb > 0 and pend_xts is not None:
                            xts_p, qb_p = pend_xts
                            for s in range(QB // P):
                                fillers += o_proj_pieces(xts_p, qb_p, s)
                            pend_xts = None
                        if kbg == 0:
                            if qb == 0:
                                ks, vs = kv_pieces(1, *kv_pre)
                                # k pieces jump the queue: the next group's
                                # scores need khT before pv needs vh
                                fillers = fillers[:2] + ks + fillers[2:] + vs
                                kv_pre = kv_dma(2)
                        if kbg == 1:
                            if qb == 0:
                                ks, vs = kv_pieces(2, *kv_pre)
                                fillers = fillers[:2] + ks + fillers[2:] + vs
                                kv_pre = kv_dma(3)
                            if qb < NQB - 1:
                                qT_next = q_dma(qb + 1)
                        # previous block's deferred p@v drains into the first
                        # key-blocks of this one (overlapping its mask latency)
                        if kbg == 2:
                            if qb == 0:
                                ks, vs = kv_pieces(3, *kv_pre)
                                fillers = fillers[:2] + ks + fillers[2:] + vs
                                nc.sync.dma_start(wo_sb[:], wo_r[:])
                            if qb < NQB - 1:
                                qhT_next, qps = q_pieces(qT_next)
                                fillers += qps
                    for _ in range(4):
                        if not carry:
                            break
                        k0, pt, j, xt_o, xts_o = carry.pop(0)
                        emit_pv(pt, k0, j, xt_o, xts_o)
                    A_sb = A_tiles[kbg]
                    ksl = slice(kb * KBLK, (kb + 1) * KBLK)

                    # scores: the two heads of a pair run concurrently in the
                    # upper/lower PE row-quadrants, into the two banks of one
                    # PSUM pair-tile.  Pair 1 (the Act-bounced one) goes first
                    # so its bank recycles with the most PE-work cover.
                    a2 = A_sb[:, i, :].unsqueeze(1).broadcast_to([P, 2, QB])
                    # pair 1 first: its Act bounce recycles the bank fastest
                    sc = psS.tile([P, 2, QB], f32, tag="scB", bufs=1, name="scB")
                    nc.tensor.matmul(
                        sc[:, 0, :], khT_sb[0:DK, 1, ksl], qhT_cur[0:DK, 1, :],
                        start=True, stop=True, tile_position=(0, 0),
                    )
                    nc.tensor.matmul(
                        sc[:, 1, :], khT_sb[DK:P, 1, ksl], qhT_cur[DK:P, 1, :],
                        start=True, stop=True, tile_position=(DK, 0),
                    )
                    mode = modes[kb]
                    ptB = sp.tile([P, 2, QB], f16, tag="ptB", bufs=10,
                                  name="ptB")
                    if mode == "dve":
                        nc.vector.tensor_tensor(ptB[:], sc[:], a2,
                                                mybir.AluOpType.mult)
                    else:
                        scb = sp.tile([P, 2, QB], f16, tag="scb", bufs=10,
                                      name="scb")
                        nc.scalar.copy(scb[:], sc[:])
                        eng = nc.gpsimd if mode == "pool" else nc.vector
                        eng.tensor_tensor(ptB[:], scb[:], a2,
                                          mybir.AluOpType.mult)
                    pendB.append((ptB, kb))
                    # pair 0: two single-bank tiles masked by two single DVE
                    # ops, issue order alternating by kb parity so each bank
                    # sees a short recycle loop every other block
                    ptA = sp.tile([P, 2, QB], f16, tag="ptA", bufs=8,
                                  name="ptA")
                    halves = []
                    for h in range(2):
                        scs1 = psS.tile([P, QB], f32, tag=("scA0", "scA1")[h],
                                        bufs=1, name=("scA0", "scA1")[h])
                        nc.tensor.matmul(
                            scs1[:], khT_sb[h * DK:(h + 1) * DK, 0, ksl],
                            qhT_cur[h * DK:(h + 1) * DK, 0, :],
                            start=True, stop=True, tile_position=(h * DK, 0),
                        )
                        halves.append(scs1)
                    order = (0, 1) if kb % 2 == 0 else (1, 0)
                    for h in order:
                        nc.vector.tensor_tensor(ptA[:, h, :], halves[h][:],
                                                A_sb[:, i, :],
                                                mybir.AluOpType.mult)
                    pendA.append((ptA, kb))

                    # software pipeline: consume pair-0/pair-1 masks several
                    # kb behind; qb0 pends extra deep, pushing p@v work out of
                    # its projection-heavy span into the next block
                    pa_max, pb_max = (6, 9) if qb == 0 else (4, 6)
                    if len(pendA) > pa_max:
                        pt, k0 = pendA.pop(0)
                        emit_pv(pt, k0, 0, xt, xts)
                    if len(pendB) > pb_max:
                        pt, k0 = pendB.pop(0)
                        emit_pv(pt, k0, 1, xt, xts)
                    budget = 1700 if qb == 0 else 420
                    spent = 0
                    while fillers and spent < budget:
                        cost, thunk = fillers.pop(0)
                        thunk()
                        spent += cost

                while fillers:
                    fillers.pop(0)[1]()
                carry = sorted(
                    [(k0, pt, 0, xt, xts) for pt, k0 in pendA] +
                    [(k0, pt, 1, xt, xts) for pt, k0 in pendB],
                    key=lambda c: (c[0], c[2]))
                pend_xts = (xts, qb)
                qhT_cur, qhT_next = qhT_next, None

            # tail: flush the last deferred p@v, then the output projection
            # cycling PSUM through the now-free score banks, drains alternating
            # between DVE and Act, and ONE merged store DMA at the end
            xts_p, qb_p = pend_xts
            for k0, pt, j, xt_o, xts_o in carry:
                emit_pv(pt, k0, j, xt_o, xts_o)
            osb_t = sp.tile([P, QB // P, D], f16, tag="osbt", bufs=1,
                            name="osb_t")
            out_r = out.rearrange("(qq p) d -> p qq d", p=P)
            po_slots = [lambda: psU.tile([P, QB], f32, tag="u", name="po"),
                        lambda: psS.tile([P, QB], f32, tag="scA0", bufs=1,
                                         name="po_a"),
                        lambda: psU.tile([P, QB], f32, tag="u", name="po"),
                        lambda: psS.tile([P, QB], f32, tag="scA1", bufs=1,
                                         name="po_b")]
            n = 0
            for ssub in range(QB // P):
                for et in range(D // QB):
                    po = po_slots[n % 4]()
                    n += 1
                    for ck in range(HD // P):
                        nc.tensor.matmul(
                            po[:], xts_p[:, ck, ssub * P:(ssub + 1) * P],
                            wo_sb[:, ck, et * QB:(et + 1) * QB],
                            start=(ck == 0), stop=(ck == HD // P - 1),
                        )
                    esl = slice(et * QB, (et + 1) * QB)
                    if et == 0:
                        nc.vector.tensor_copy(osb_t[:, ssub, esl], po[:])
                    else:
                        nc.scalar.copy(osb_t[:, ssub, esl], po[:])
                if ssub % 2 == 1:
                    q0 = qb_p * (QB // P) + ssub - 1
                    nc.sync.dma_start(out_r[:, q0:q0 + 2, :],
                                      osb_t[:, ssub - 1:ssub + 1, :])

    nc.compile()
    return nc


def _numpy_fallback(q, k, v, A, Wq, bq, Wk, bk, Wv, bv, Wo, bo):
    def proj(x, W, b):
        y = x @ W.T + b
        return y.reshape(B, S, H, DK).transpose(0, 2, 1, 3)

    qh, kh, vh = proj(q, Wq, bq), proj(k, Wk, bk), proj(v, Wv, bv)
    scores = np.einsum("bhqd,bhkd->bhqk", qh, kh) * np.float32(SCALE)
    p = scores * A.T
    x = np.einsum("bhqk,bhkd->bhqd", p, vh)
    x = x.transpose(0, 2, 1, 3).reshape(B, S, D)
    return (x @ Wo.T + bo).astype(np.float32)


def kernel(**inputs):
    q = np.asarray(inputs["q"], dtype=np.float32)
    k = np.asarray(inputs["k"], dtype=np.float32)
    v = np.asarray(inputs["v"], dtype=np.float32)
    A = np.asarray(inputs["A"], dtype=np.float32)
    Wq = np.asarray(inputs["Wq"], dtype=np.float32)
    Wk = np.asarray(inputs["Wk"], dtype=np.float32)
    Wv = np.asarray(inputs["Wv"], dtype=np.float32)
    Wo = np.asarray(inputs["Wo"], dtype=np.float32)
    bq, bk, bv, bo = (np.asarray(inputs[n], dtype=np.float32) for n in ("bq", "bk", "bv", "bo"))

    # The device kernel folds zero biases away (spec fills them with zeros);
    # fall back to a host reference in the (unused) nonzero-bias case.
    if any(np.any(b) for b in (bq, bk, bv)):
        return _numpy_fallback(q, k, v, A, Wq, bq, Wk, bk, Wv, bv, Wo, bo)

    global _CACHED
    if _CACHED is None:
        _CACHED = _build()
    nc = _CACHED

    Asc = np.ascontiguousarray((A * np.float32(SCALE)).astype(np.float16))
    in_maps = []
    for c in range(NCORES):
        b, g = divmod(c, GROUPS)
        hsl = slice(g * HD, (g + 1) * HD)
        in_maps.append({
            "qT": np.ascontiguousarray(q[b].T.astype(np.float16)),
            "kT": np.ascontiguousarray(k[b].T.astype(np.float16)),
            "vT": np.ascontiguousarray(v[b].T.astype(np.float16)),
            "Asc": Asc,
            "wq": np.ascontiguousarray(Wq[hsl].T.astype(np.float16)),
            "wk": np.ascontiguousarray(Wk[hsl].T.astype(np.float16)),
            "wv": np.ascontiguousarray(Wv[hsl].T.astype(np.float16)),
            "wo": np.ascontiguousarray(Wo[:, hsl].T),
        })

    res = bass_utils.run_bass_kernel_spmd(
        nc, in_maps, core_ids=list(range(NCORES)), trace=TRACE
    )
    global LAST_RESULTS
    LAST_RESULTS = res

    out = np.zeros((B, S, D), dtype=np.float32)
    for c in range(NCORES):
        out[c // GROUPS] += res.results[c]["out"].astype(np.float32)
    out += bo
    return out


if __name__ == "__main__":
    rng = np.random.default_rng(0)
    ins = {
        "q": rng.standard_normal((B, S, D), dtype=np.float32),
        "k": rng.standard_normal((B, S, D), dtype=np.float32),
        "v": rng.standard_normal((B, S, D), dtype=np.float32),
        "A": rng.random((S, S), dtype=np.float32),
        "Wq": rng.standard_normal((D, D), dtype=np.float32) / 32,
        "bq": np.zeros(D, np.float32),
        "Wk": rng.standard_normal((D, D), dtype=np.float32) / 32,
        "bk": np.zeros(D, np.float32),
        "Wv": rng.standard_normal((D, D), dtype=np.float32) / 32,
        "bv": np.zeros(D, np.float32),
        "Wo": rng.standard_normal((D, D), dtype=np.float32) / 32,
        "bo": np.zeros(D, np.float32),
    }
    got = kernel(**ins)
    ref = _numpy_fallback(**ins)
    err = np.abs(got - ref).max() / np.abs(ref).max()
    print("self-check relmax:", err)


# revision 39
# speedup vs baseline: 1.1791x; 1.0174x over previous
"""Trainium2 Bass kernel for nn_AttentionBlock (sparse_attention, no-softmax).

Computation (per batch b):
    qh = (q @ Wq^T) split into 16 heads of dk=64     [S, D] -> [H, S, DK]
    kh, vh likewise
    scores = (qh @ kh^T) / sqrt(DK)                  [H, S, S]
    p      = scores * A^T                            (elementwise structural mask)
    x      = p @ vh                                  [H, S, DK] -> [S, D]
    out    = x @ Wo^T + bo                           [S, D]

Sharding over 8 NeuronCores: data-parallel over batch (B=2) x tensor-parallel
over heads (16 heads -> 4 per core). Each core projects q/k/v for its 4 heads
(column-parallel), runs masked attention for them, and applies its 256-column
slice of the output projection (row-parallel), producing a full-shape partial
output. Host sums the 4 partials per batch.

Implementation notes:
- Activations ship pre-transposed ([D, S]); 1/sqrt(DK) is folded into the
  mask A on the host; whole data path in fp16 with fp32 PSUM accumulation.
- Heads are stored as pairs on the partition axis. Score matmuls of a pair
  run concurrently in the upper/lower PE row-quadrants into the TWO banks of
  one [128,2,512] PSUM tile; p@v matmuls run concurrently in left/right
  col-quadrants of one bank.
- The mask multiply processes a head-pair per instruction ([128,2,512], the
  A block shared across the pair via a stride-0 broadcast AP). Work is split
  three ways to fit under the PE time: DVE straight out of PSUM, an
  Activation PSUM->SBUF fp16 bounce feeding either GPSIMD or a 2x-mode DVE
  multiply (all-fp16-SBUF operands run at 2 elem/cycle on DVE).
- p@v consumption is software-pipelined 2 key-blocks behind the DVE-masked
  pair and 4 key-blocks behind the bounced pairs, hiding mask latency.
- Projection work for other blocks is interleaved into the attention loop;
  output is stored fp16 (partials summed on host in fp32).
"""

import numpy as np

import concourse.mybir as mybir
import concourse.tile as tile
from concourse import bacc, bass_utils

B, S, D, H = 2, 2048, 1024, 16
NCORES = 8
GROUPS = NCORES // B          # 4 head-groups
HPC = H // GROUPS             # 4 heads per core
DK = D // H                   # 64
HD = HPC * DK                 # 256 head-dim columns per core
NPAIR = HPC // 2              # 2 head pairs per core
SCALE = 1.0 / np.sqrt(DK)

P = 128                       # SBUF partitions
QB = 512                      # query block
NQB = S // QB                 # 4
KBLK = 128                    # key block
NKB = S // KBLK               # 16
NKT = D // P                  # 8 contraction chunks for projections
AGRP = 4                      # key-blocks per A-tile DMA
NGRP = NKB // AGRP            # 4 groups

f32 = mybir.dt.float32
f16 = mybir.dt.float16
f32r = mybir.dt.float32r

# per-kb mask path for the second head pair (first pair always DVE-from-PSUM).
# 'pool': Act bounce -> GPSIMD;  'dve2x': Act bounce -> DVE 2x;  'dve': DVE.
# The pool path has the longest latency, so it is confined to early key-blocks
# and its p@v consumption pended the deepest; late key-blocks use the fast
# paths so the end-of-block pipeline flush never waits on GPSIMD.
MODES_STEADY = ["pool", "dve2x", "pool", "pool",
                "pool", "dve2x", "pool", "pool",
                "pool", "pool", "pool", "pool",
                "dve2x", "pool", "pool", "pool"]
# qb0's PE span is projection-heavy (all of K/V): more slack for bounces
MODES_QB0 = ["pool", "pool", "pool", "pool",
             "pool", "pool", "dve2x", "pool",
             "pool", "pool", "pool", "pool",
             "pool", "pool", "pool", "dve"]
# last block: no pool in the final key-blocks so the tail flush+output
# projection never waits on GPSIMD latency
MODES_LAST = ["pool", "pool", "pool", "pool",
              "pool", "pool", "pool", "pool",
              "pool", "pool", "pool", "dve2x",
              "dve2x", "dve2x", "dve2x", "dve2x"]

_CACHED = None  # built module, reused across kernel() calls
TRACE = False         # set True (e.g. from test.py) to profile the NEFF
LAST_RESULTS = None   # BassKernelResults of the most recent run


def _build():
    nc = bacc.Bacc("TRN2", target_bir_lowering=False)

    qT = nc.dram_tensor("qT", [D, S], f16, kind="ExternalInput")
    kT = nc.dram_tensor("kT", [D, S], f16, kind="ExternalInput")
    vT = nc.dram_tensor("vT", [D, S], f16, kind="ExternalInput")
    Asc = nc.dram_tensor("Asc", [S, S], f16, kind="ExternalInput")
    wq = nc.dram_tensor("wq", [D, HD], f16, kind="ExternalInput")
    wk = nc.dram_tensor("wk", [D, HD], f16, kind="ExternalInput")
    wv = nc.dram_tensor("wv", [D, HD], f16, kind="ExternalInput")
    wo = nc.dram_tensor("wo", [HD, D], f32r, kind="ExternalInput")
    out = nc.dram_tensor("out", [S, D], f16, kind="ExternalOutput")

    qT_r = qT.rearrange("(kt p) s -> p kt s", p=P)
    kT_r = kT.rearrange("(kt p) s -> p kt s", p=P)
    vT_r = vT.rearrange("(kt p) s -> p kt s", p=P)
    wq_r = wq.rearrange("(kt p) c -> p kt c", p=P)
    wk_r = wk.rearrange("(kt p) c -> p kt c", p=P)
    wv_r = wv.rearrange("(kt p) c -> p kt c", p=P)
    wo_r = wo.rearrange("(ck p) e -> p ck e", p=P)
    A_r = Asc.rearrange("(kb p) q -> p kb q", p=P)

    with tile.TileContext(nc) as tc:
        with (
            tc.tile_pool(name="persist", bufs=1) as pp,
            tc.tile_pool(name="stream", bufs=2) as sp,
            tc.tile_pool(name="psU", bufs=2, space="PSUM") as psU,   # proj/oproj
            tc.tile_pool(name="psS", bufs=1, space="PSUM") as psS,   # score pairs
            tc.tile_pool(name="psX", bufs=1, space="PSUM") as psX,   # xT accum
        ):
            wk_sb = pp.tile([P, NKT, HD], f16, tag="wk")
            wv_sb = pp.tile([P, NKT, HD], f16, tag="wv")
            wq_sb = pp.tile([P, NKT, HD], f16, tag="wq")
            wo_sb = pp.tile([P, HD // P, D], f32r, tag="wo")

            # head-PAIR layout: pair j holds head 2j on partitions 0:64 and
            # head 2j+1 on 64:128
            khT_sb = pp.tile([P, NPAIR, S], f16, tag="khT")
            vh_sb = pp.tile([P, NKB, HD], f16, tag="vh")     # [ks%128, kb, c]

            # ---- interleavable projection work, split into ~850ns pieces ----
            # each piece is (cost_ns, thunk); PSUM tiles are allocated lazily
            # by the first piece of a chain and carried in a cell

            def kchain_pieces(ct, kT_sb, st):
                sl = slice(st * QB, (st + 1) * QB)
                cell = {}
                def half(h):
                    if h == 0:
                        cell["pk"] = psU.tile([P, QB], f32, tag="u", name="pk")
                    pk = cell["pk"]
                    for kt in range(h * NKT // 2, (h + 1) * NKT // 2):
                        nc.tensor.matmul(
                            pk[:], wk_sb[:, kt, ct * P:(ct + 1) * P], kT_sb[:, kt, :],
                            start=(kt == 0), stop=(kt == NKT - 1),
                        )
                    if h == 1:
                        nc.scalar.copy(khT_sb[:, ct, sl], pk[:])
                return [(853, lambda h=h: half(h)) for h in range(2)]

            def vchain_pieces(ssub, vT_sb, st):
                kb = st * (QB // P) + ssub
                def run():
                    pv = psU.tile([P, HD], f32, tag="u", name="pv")
                    for kt in range(NKT):
                        nc.tensor.matmul(
                            pv[:], vT_sb[:, kt, ssub * P:(ssub + 1) * P], wv_sb[:, kt, :],
                            start=(kt == 0), stop=(kt == NKT - 1),
                        )
                    nc.scalar.copy(vh_sb[:, kb, :], pv[:])
                return [(853, run)]

            def kv_dma(st):
                sl = slice(st * QB, (st + 1) * QB)
                kT_sb = sp.tile([P, NKT, QB], f16, tag="xin", bufs=7, name="kT_sb")
                nc.sync.dma_start(kT_sb[:], kT_r[:, :, sl])
                vT_sb = sp.tile([P, NKT, QB], f16, tag="xin", bufs=7, name="vT_sb")
                nc.sync.dma_start(vT_sb[:], vT_r[:, :, sl])
                return kT_sb, vT_sb

            def kv_pieces(st, kT_sb, vT_sb):
                ks = kchain_pieces(0, kT_sb, st) + kchain_pieces(1, kT_sb, st)
                vs = []
                for ss in range(QB // P):
                    vs += vchain_pieces(ss, vT_sb, st)
                return ks, vs

            def q_dma(qb):
                qsl = slice(qb * QB, (qb + 1) * QB)
                qT_sb = sp.tile([P, NKT, QB], f16, tag="xin", bufs=7, name="qT_sb")
                nc.sync.dma_start(qT_sb[:], qT_r[:, :, qsl])
                return qT_sb

            def qchain_pieces(ct, qT_sb, qhT_sb):
                cell = {}
                def half(h):
                    if h == 0:
                        cell["pq"] = psU.tile([P, QB], f32, tag="u", name="pq")
                    pq = cell["pq"]
                    for kt in range(h * NKT // 2, (h + 1) * NKT // 2):
                        nc.tensor.matmul(
                            pq[:], wq_sb[:, kt, ct * P:(ct + 1) * P], qT_sb[:, kt, :],
                            start=(kt == 0), stop=(kt == NKT - 1),
                        )
                    if h == 1:
                        nc.scalar.copy(qhT_sb[:, ct, :], pq[:])
                return [(853, lambda h=h: half(h)) for h in range(2)]

            def q_pieces(qT_sb):
                qhT_sb = sp.tile([P, NPAIR, QB], f16, tag="qh", bufs=3, name="qhT_sb")
                ps = qchain_pieces(0, qT_sb, qhT_sb) + qchain_pieces(1, qT_sb, qhT_sb)
                return qhT_sb, ps

            def o_proj_pieces(xts, qb, ssub):
                """One 128-row slice of the output projection: one piece per
                512-column chunk (matmuls + PSUM drain), then the store DMA as
                its own zero-cost piece — by the time it is dispatched the
                drains are done, so it never head-of-line blocks the SP
                queue."""
                osb = sp.tile([P, D], f16, tag="osb", bufs=4, name="osb")
                rsl = slice(qb * QB + ssub * P, qb * QB + (ssub + 1) * P)
                def piece(et):
                    po = psU.tile([P, QB], f32, tag="u", name="po")
                    for ck in range(HD // P):
                        nc.tensor.matmul(
                            po[:],
                            xts[:, ck, ssub * P:(ssub + 1) * P],
                            wo_sb[:, ck, et * QB:(et + 1) * QB],
                            start=(ck == 0), stop=(ck == HD // P - 1),
                        )
                    esl = slice(et * QB, (et + 1) * QB)
                    nc.scalar.copy(osb[:, esl], po[:])
                return [(427, lambda et=et: piece(et)) for et in range(D // QB)] + \
                       [(0, lambda: nc.sync.dma_start(out[rsl, :], osb[:]))]

            def o_proj_chain(xts, qb, ssub):
                for _, t in o_proj_pieces(xts, qb, ssub):
                    t()

            # ---- prologue DMAs: wk/kT0 interleaved halves so the first
            # k-chain piece starts as soon as possible
            kT0 = sp.tile([P, NKT, QB], f16, tag="xin", bufs=7, name="kT_sb")
            h1 = slice(0, NKT // 2)
            h2 = slice(NKT // 2, NKT)
            nc.sync.dma_start(wk_sb[:, h1, :], wk_r[:, h1, :])
            nc.sync.dma_start(kT0[:, 0:2, :], kT_r[:, 0:2, 0:QB])
            nc.sync.dma_start(kT0[:, 2:4, :], kT_r[:, 2:4, 0:QB])
            nc.sync.dma_start(wq_sb[:], wq_r[:])
            nc.sync.dma_start(wk_sb[:, h2, :], wk_r[:, h2, :])
            nc.sync.dma_start(kT0[:, h2, :], kT_r[:, h2, 0:QB])
            qT0 = q_dma(0)
            nc.sync.dma_start(wv_sb[:], wv_r[:])
            vT0 = sp.tile([P, NKT, QB], f16, tag="xin", bufs=7, name="vT_sb")
            nc.sync.dma_start(vT0[:], vT_r[:, :, 0:QB])
            A0 = sp.tile([P, AGRP, QB], f16, tag="A", bufs=4, name="A_sb")
            nc.sync.dma_start(A0[:], A_r[:, 0:AGRP, 0:QB])

            # prologue compute: k and q projections first so attention can
            # start; first halves of both k chains run back-to-back so the
            # second halves never outrun the second DMA chunk
            k0 = kchain_pieces(0, kT0, 0)
            k1 = kchain_pieces(1, kT0, 0)
            qhT_cur, qps = q_pieces(qT0)
            for _, t in [k0[0], k1[0], k0[1], k1[1]] + qps:
                t()
            kv_pre = kv_dma(1)

            fillers = []
            for ss in range(QB // P):
                fillers += vchain_pieces(ss, vT0, 0)

            pend_xts = None    # (xts tile, qb) awaiting output projection
            qhT_next = None
            qT_next = None
            A_next = None      # next qb's prefetched A tiles {g: tile}
            carry = []         # previous qb's unconsumed (kb, pt, j, xt, xts)

            def emit_pv(pt, kb, j, xt, xts):
                # p @ v: both heads of a pair run concurrently in the
                # left/right PE col-quadrants into one PSUM bank.
                nc.tensor.matmul(
                    xt[0:DK, j, :],
                    vh_sb[:, kb, (2 * j) * DK:(2 * j + 1) * DK],
                    pt[:, 0, :],
                    start=(kb == 0), stop=(kb == NKB - 1),
                    tile_position=(0, 0), skip_group_check=True,
                )
                nc.tensor.matmul(
                    xt[DK:P, j, :],
                    vh_sb[:, kb, (2 * j + 1) * DK:(2 * j + 2) * DK],
                    pt[:, 1, :],
                    start=(kb == 0), stop=(kb == NKB - 1),
                    tile_position=(0, DK), skip_group_check=True,
                )
                if kb == NKB - 1:
                    nc.scalar.copy(xts[:, j, :], xt[:, j, :])

            for qb in range(NQB):
                qsl = slice(qb * QB, (qb + 1) * QB)
                if qb == 0:
                    modes = MODES_QB0
                elif qb == NQB - 1:
                    modes = MODES_LAST
                else:
                    modes = MODES_STEADY
                xt = psX.tile([P, NPAIR, QB], f32, tag="xt", name="xt")  # 2 banks
                xts = sp.tile([P, NPAIR, QB], f32r, tag="xts", bufs=3, name="xts")
                pendA = []   # (pt tile, kb) for pair 0, consumed kb+3
                pendB = []   # (pt tile, kb) for pair 1, consumed kb+5

                def a_dma(g, qb_of):
                    t = sp.tile([P, AGRP, QB], f16, tag="A", bufs=4, name="A_sb")
                    nc.sync.dma_start(
                        t[:], A_r[:, g * AGRP:(g + 1) * AGRP,
                                  qb_of * QB:(qb_of + 1) * QB])
                    return t

                if qb == 0:
                    A_tiles = {0: A0, 1: a_dma(1, 0)}
                else:
                    A_tiles = A_next          # prefetched during previous qb
                A_next = {}

                for kb in range(NKB):
                    kbg, i = divmod(kb, AGRP)
                    if i == 0:
                        # A prefetch, two groups ahead (wraps into next qb)
                        tgt = kbg + 2
                        if tgt < NGRP:
                            A_tiles[tgt] = a_dma(tgt, qb)
                        elif qb < NQB - 1:
                            A_next[tgt - NGRP] = a_dma(tgt - NGRP, qb + 1)
                        if kbg == 1 and qb > 0 and pend_xts is not None:
                            xts_p, qb_p = pend_xts
                            for s in range(QB // P):
                                fillers += o_proj_pieces(xts_p, qb_p, s)
                            pend_xts = None
                        if kbg == 0:
                            if qb == 0:
                                ks, vs = kv_pieces(1, *kv_pre)
                                # k pieces jump the queue: the next group's
                                # scores need khT before pv needs vh
                                fillers = fillers[:2] + ks + fillers[2:] + vs
                                kv_pre = kv_dma(2)
                        if kbg == 1:
                            if qb == 0:
                                ks, vs = kv_pieces(2, *kv_pre)
                                fillers = fillers[:2] + ks + fillers[2:] + vs
                                kv_pre = kv_dma(3)
                            if qb < NQB - 1:
                                qT_next = q_dma(qb + 1)
                        # previous block's deferred p@v drains into the first
                        # key-blocks of this one (overlapping its mask latency)
                        if kbg == 2:
                            if qb == 0:
                                ks, vs = kv_pieces(3, *kv_pre)
                                fillers = fillers[:2] + ks + fillers[2:] + vs
                                nc.sync.dma_start(wo_sb[:], wo_r[:])
                            if qb < NQB - 1:
                                qhT_next, qps = q_pieces(qT_next)
                                fillers += qps
                    for _ in range(4):
                        if not carry:
                            break
                        k0, pt, j, xt_o, xts_o = carry.pop(0)
                        emit_pv(pt, k0, j, xt_o, xts_o)
                    A_sb = A_tiles[kbg]
                    ksl = slice(kb * KBLK, (kb + 1) * KBLK)

                    # scores: the two heads of a pair run concurrently in the
                    # upper/lower PE row-quadrants, into the two banks of one
                    # PSUM pair-tile.  Pair 1 (the Act-bounced one) goes first
                    # so its bank recycles with the most PE-work cover.
                    a2 = A_sb[:, i, :].unsqueeze(1).broadcast_to([P, 2, QB])
                    # pair 1 first: its Act bounce recycles the bank fastest
                    sc = psS.tile([P, 2, QB], f32, tag="scB", bufs=1, name="scB")
                    nc.tensor.matmul(
                        sc[:, 0, :], khT_sb[0:DK, 1, ksl], qhT_cur[0:DK, 1, :],
                        start=True, stop=True, tile_position=(0, 0),
                    )
                    nc.tensor.matmul(
                        sc[:, 1, :], khT_sb[DK:P, 1, ksl], qhT_cur[DK:P, 1, :],
                        start=True, stop=True, tile_position=(DK, 0),
                    )
                    mode = modes[kb]
                    ptB = sp.tile([P, 2, QB], f16, tag="ptB", bufs=12,
                                  name="ptB")
                    if mode == "dve":
                        nc.vector.tensor_tensor(ptB[:], sc[:], a2,
                                                mybir.AluOpType.mult)
                    else:
                        scb = sp.tile([P, 2, QB], f16, tag="scb", bufs=11,
                                      name="scb")
                        nc.scalar.copy(scb[:], sc[:])
                        eng = nc.gpsimd if mode == "pool" else nc.vector
                        eng.tensor_tensor(ptB[:], scb[:], a2,
                                          mybir.AluOpType.mult)
                    pendB.append((ptB, kb))
                    # pair 0: two single-bank tiles masked by two single DVE
                    # ops, issue order alternating by kb parity so each bank
                    # sees a short recycle loop every other block
                    ptA = sp.tile([P, 2, QB], f16, tag="ptA", bufs=9,
                                  name="ptA")
                    halves = []
                    for h in range(2):
                        scs1 = psS.tile([P, QB], f32, tag=("scA0", "scA1")[h],
                                        bufs=1, name=("scA0", "scA1")[h])
                        nc.tensor.matmul(
                            scs1[:], khT_sb[h * DK:(h + 1) * DK, 0, ksl],
                            qhT_cur[h * DK:(h + 1) * DK, 0, :],
                            start=True, stop=True, tile_position=(h * DK, 0),
                        )
                        halves.append(scs1)
                    order = (0, 1) if kb % 2 == 0 else (1, 0)
                    for h in order:
                        nc.vector.tensor_tensor(ptA[:, h, :], halves[h][:],
                                                A_sb[:, i, :],
                                                mybir.AluOpType.mult)
                    pendA.append((ptA, kb))

                    # software pipeline: consume pair-0/pair-1 masks several
                    # kb behind; qb0 pends extra deep, pushing p@v work out of
                    # its projection-heavy span into the next block
                    pa_max, pb_max = [(7, 10), (4, 6), (4, 6), (3, 5)][qb]
                    if len(pendA) > pa_max:
                        pt, k0 = pendA.pop(0)
                        emit_pv(pt, k0, 0, xt, xts)
                    if len(pendB) > pb_max:
                        pt, k0 = pendB.pop(0)
                        emit_pv(pt, k0, 1, xt, xts)
                    budget = 1700 if qb == 0 else 420
                    spent = 0
                    while fillers and spent < budget:
                        cost, thunk = fillers.pop(0)
                        thunk()
                        spent += cost

                while fillers:
                    fillers.pop(0)[1]()
                carry = sorted(
                    [(k0, pt, 0, xt, xts) for pt, k0 in pendA] +
                    [(k0, pt, 1, xt, xts) for pt, k0 in pendB],
                    key=lambda c: (c[0], c[2]))
                pend_xts = (xts, qb)
                qhT_cur, qhT_next = qhT_next, None

            # tail: flush the last deferred p@v, then the output projection
            # cycling PSUM through the now-free score banks, drains alternating
            # between DVE and Act, and ONE merged store DMA at the end
            xts_p, qb_p = pend_xts
            for k0, pt, j, xt_o, xts_o in carry:
                emit_pv(pt, k0, j, xt_o, xts_o)
            osb_t = sp.tile([P, QB // P, D], f16, tag="osbt", bufs=1,
                            name="osb_t")
            out_r = out.rearrange("(qq p) d -> p qq d", p=P)
            po_slots = [lambda: psU.tile([P, QB], f32, tag="u", name="po"),
                        lambda: psS.tile([P, QB], f32, tag="scA0", bufs=1,
                                         name="po_a"),
                        lambda: psU.tile([P, QB], f32, tag="u", name="po"),
                        lambda: psS.tile([P, QB], f32, tag="scA1", bufs=1,
                                         name="po_b")]
            n = 0
            for ssub in range(QB // P):
                for et in range(D // QB):
                    po = po_slots[n % 4]()
                    n += 1
                    for ck in range(HD // P):
                        nc.tensor.matmul(
                            po[:], xts_p[:, ck, ssub * P:(ssub + 1) * P],
                            wo_sb[:, ck, et * QB:(et + 1) * QB],
                            start=(ck == 0), stop=(ck == HD // P - 1),
                        )
                    esl = slice(et * QB, (et + 1) * QB)
                    if et == 0:
                        nc.vector.tensor_copy(osb_t[:, ssub, esl], po[:])
                    else:
                        nc.scalar.copy(osb_t[:, ssub, esl], po[:])
                q0 = qb_p * (QB // P) + ssub
                nc.sync.dma_start(out_r[:, q0:q0 + 1, :],
                                  osb_t[:, ssub:ssub + 1, :])

    nc.compile()
    return nc


def _numpy_fallback(q, k, v, A, Wq, bq, Wk, bk, Wv, bv, Wo, bo):
    def proj(x, W, b):
        y = x @ W.T + b
        return y.reshape(B, S, H, DK).transpose(0, 2, 1, 3)

    qh, kh, vh = proj(q, Wq, bq), proj(k, Wk, bk), proj(v, Wv, bv)
    scores = np.einsum("bhqd,bhkd->bhqk", qh, kh) * np.float32(SCALE)
    p = scores * A.T
    x = np.einsum("bhqk,bhkd->bhqd", p, vh)
    x = x.transpose(0, 2, 1, 3).reshape(B, S, D)
    return (x @ Wo.T + bo).astype(np.float32)


def kernel(**inputs):
    q = np.asarray(inputs["q"], dtype=np.float32)
    k = np.asarray(inputs["k"], dtype=np.float32)
    v = np.asarray(inputs["v"], dtype=np.float32)
    A = np.asarray(inputs["A"], dtype=np.float32)
    Wq = np.asarray(inputs["Wq"], dtype=np.float32)
    Wk = np.asarray(inputs["Wk"], dtype=np.float32)
    Wv = np.asarray(inputs["Wv"], dtype=np.float32)
    Wo = np.asarray(inputs["Wo"], dtype=np.float32)
    bq, bk, bv, bo = (np.asarray(inputs[n], dtype=np.float32) for n in ("bq", "bk", "bv", "bo"))

    # The device kernel folds zero biases away (spec fills them with zeros);
    # fall back to a host reference in the (unused) nonzero-bias case.
    if any(np.any(b) for b in (bq, bk, bv)):
        return _numpy_fallback(q, k, v, A, Wq, bq, Wk, bk, Wv, bv, Wo, bo)

    global _CACHED
    if _CACHED is None:
        _CACHED = _build()
    nc = _CACHED

    Asc = np.ascontiguousarray((A * np.float32(SCALE)).astype(np.float16))
    in_maps = []
    for c in range(NCORES):
        b, g = divmod(c, GROUPS)
        hsl = slice(g * HD, (g + 1) * HD)
        in_maps.append({
            "qT": np.ascontiguousarray(q[b].T.astype(np.float16)),
            "kT": np.ascontiguousarray(k[b].T.astype(np.float16)),
            "vT": np.ascontiguousarray(v[b].T.astype(np.float16)),
            "Asc": Asc,
            "wq": np.ascontiguousarray(Wq[hsl].T.astype(np.float16)),
            "wk": np.ascontiguousarray(Wk[hsl].T.astype(np.float16)),
            "wv": np.ascontiguousarray(Wv[hsl].T.astype(np.float16)),
            "wo": np.ascontiguousarray(Wo[:, hsl].T),
        })

    res = bass_utils.run_bass_kernel_spmd(
        nc, in_maps, core_ids=list(range(NCORES)), trace=TRACE
    )
    global LAST_RESULTS
    LAST_RESULTS = res

    out = np.zeros((B, S, D), dtype=np.float32)
    for c in range(NCORES):
        out[c // GROUPS] += res.results[c]["out"].astype(np.float32)
    out += bo
    return out


if __name__ == "__main__":
    rng = np.random.default_rng(0)
    ins = {
        "q": rng.standard_normal((B, S, D), dtype=np.float32),
        "k": rng.standard_normal((B, S, D), dtype=np.float32),
        "v": rng.standard_normal((B, S, D), dtype=np.float32),
        "A": rng.random((S, S), dtype=np.float32),
        "Wq": rng.standard_normal((D, D), dtype=np.float32) / 32,
        "bq": np.zeros(D, np.float32),
        "Wk": rng.standard_normal((D, D), dtype=np.float32) / 32,
        "bk": np.zeros(D, np.float32),
        "Wv": rng.standard_normal((D, D), dtype=np.float32) / 32,
        "bv": np.zeros(D, np.float32),
        "Wo": rng.standard_normal((D, D), dtype=np.float32) / 32,
        "bo": np.zeros(D, np.float32),
    }
    got = kernel(**ins)
    ref = _numpy_fallback(**ins)
    err = np.abs(got - ref).max() / np.abs(ref).max()
    print("self-check relmax:", err)
